# revision 50
# baseline (speedup 1.0000x reference)
"""DiffAttention (nn_DiffAttention) — Trainium2 Bass kernel, 8 NeuronCores.

Sharding: 4 batches x 6 effective heads = 24 units -> core c gets batch
c//2 and effective heads [3*(c%2), 3*(c%2)+3).  Each core computes its
q/k/v projections (column-sliced), both softmax maps per eff-head, the
differential combination, head RMS-norm, and its partial output
projection; the host sums the two per-batch partials and adds bo.

All matmuls run in bf16 (fp32 PSUM accumulation).  Softmax is computed
without max-subtraction (scores are O(5) here) and without explicit
normalization: the softmax denominators d1, d2 enter through
lamp = lambda*d1/d2 and the eps-correction of the scale-invariant
RMS norm (rms(c*u) ~ c*rms(u)):

  E1 = exp(S1), E2 = exp(S2)         (ACT, accum_out -> d1, d2)
  diffn = lamp*E2 - E1               (DVE)
  u = diffn @ V                      (PE; ref out_pre = -(1/d1)*u)
  r = rsqrt(mean_dv(u^2) + eps*d1^2) (DVE + ACT ln/exp)
  out_n = u*r*(-0.2*wn)              (signs/scales cancel exactly)
  y += out_n @ Wo_slice              (PE)

ACT (exp: 192 calls of [128,1024], ~1.2us each) is the critical engine;
the schedule keeps its queue free of everything except exps and the
late-emitted, batched rms Ln/Exp tail calls.
"""

import os
import sys
from contextlib import ExitStack

import numpy as np

try:
    import concourse.bass as bass  # noqa: F401
except ImportError:
    for _p in ("/opt/trn_rl_repo", os.path.expanduser("~/trn_rl_repo")):
        if os.path.isdir(_p):
            sys.path.insert(0, _p)
            break
    import concourse.bass as bass  # noqa: F401

import ml_dtypes
import concourse.tile as tile
from concourse import bacc, bass_utils, mybir
from concourse.bass import ts

F32 = mybir.dt.float32
BF16 = mybir.dt.bfloat16
AF = mybir.ActivationFunctionType
OP = mybir.AluOpType

B = 4
N = 2048
D = 768
HD = 64
EH = 3  # eff heads per core
NT = N // 128
NCORES = 8
EPS = 1e-5
LAMBDA_INIT = 0.8
BF = ml_dtypes.bfloat16


def _body(ctx, tc, xT, wq, wk, wv, wo, bq, bk, bv128, lam128, y):
    nc = tc.nc

    const = ctx.enter_context(tc.tile_pool(name="const", bufs=1))
    wpool = ctx.enter_context(tc.tile_pool(name="wpool", bufs=1))
    xpool = ctx.enter_context(tc.tile_pool(name="xpool", bufs=1))
    qkv = ctx.enter_context(tc.tile_pool(name="qkv", bufs=1))
    # psum: scores 3x2 banks (one tile, manually rotated ring so adjacent
    # slots can merge into single [128,2048] exp calls) + pv/proj 2 = 8
    spool = ctx.enter_context(tc.tile_pool(name="spool", bufs=1, space="PSUM"))
    pvp = ctx.enter_context(tc.tile_pool(name="pvp", bufs=1, space="PSUM"))
    epool = ctx.enter_context(tc.tile_pool(name="epool", bufs=8))
    dpool = ctx.enter_context(tc.tile_pool(name="dpool", bufs=4))
    dtp = ctx.enter_context(tc.tile_pool(name="dtp", bufs=2))
    small = ctx.enter_context(tc.tile_pool(name="small", bufs=4))
    onp = ctx.enter_context(tc.tile_pool(name="onp", bufs=3))
    ysb = ctx.enter_context(tc.tile_pool(name="ysb", bufs=2))

    # ---- input DMAs -----------------------------------------------------
    # Startup is bound by per-queue serial DMA time (~0.39ns/B/partition),
    # so the exp1-critical pieces (x cols 0:1024, wk, wq) are split across
    # the sync and gpsimd queues so both finish ~6us; x cols 1024:2048
    # follow on both queues' tails.  The ACT queue issues no DMAs at all:
    # it must stay free for the exp stream.
    bq_t = const.tile([128, 3], F32)
    bk_t = const.tile([128, 3], F32)
    # warm the PE p-state ramp (full clock needs ~3us from first busy)
    # with zero matmuls so the first projection runs at 2.4GHz, not 0.65
    zpe = const.tile([128, 512], BF16)
    nc.vector.memset(zpe[:], 0)
    # warm the ACT exp/ln table while DMAs stream (first real exp would
    # otherwise pay the ~2.7us table load on the critical path)
    actwarm = const.tile([128, 1], F32)
    nc.scalar.activation(actwarm[:], zpe[:, 0:1], AF.Exp)
    nc.scalar.activation(actwarm[:], actwarm[:], AF.Ln)
    for _ in range(7):
        psz = pvp.tile([128, 512], F32, tag="pv", name="psz")
        nc.tensor.matmul(psz[:], zpe[:, 0:128], zpe[:], start=True, stop=True)
    xt = xpool.tile([128, 6, N], BF16, tag="xt", name="xt")

    def x_dma(eng, c0, c1):
        eng.dma_start(
            xt[:, :, c0:c1],
            xT[:, c0:c1].rearrange("(a p) c -> p a c", p=128),
        )

    # ACT's queue is idle until the first exp (~8us), so it carries the
    # tiny biases and one x quarter; nothing may queue on it after that
    x_dma(nc.sync, 0, 512)
    nc.scalar.dma_start(bk_t[:], bk)
    nc.scalar.dma_start(bq_t[:], bq)
    nc.scalar.dma_start(
        xt[:, :, 1024:1536],
        xT[:, 1024:1536].rearrange("(a p) c -> p a c", p=128),
    )
    w_t = {}
    for name, ap in (("k", wk), ("q", wq), ("v", wv)):
        tiles = []
        for i in range(6):
            t = wpool.tile([128, 384], BF16, tag=f"w{name}{i}", name=f"w{name}{i}")
            tiles.append(t)
        w_t[name] = tiles
    for i in range(6):
        nc.gpsimd.dma_start(w_t["k"][i][:], wk[ts(i, 128), :])
    x_dma(nc.sync, 512, 1024)
    for i in range(6):
        nc.gpsimd.dma_start(w_t["q"][i][:], wq[ts(i, 128), :])
    x_dma(nc.sync, 1536, 2048)
    lam_t = const.tile([128, 1], F32)
    nc.sync.dma_start(lam_t[:], lam128)
    bv_t = const.tile([128, 384], F32)
    nc.sync.dma_start(bv_t[:], bv128)
    wo_t = []
    for h in range(EH):
        t = wpool.tile([128, D], BF16, tag=f"wo{h}", name=f"wo{h}")
        wo_t.append(t)

    def late_w_dmas():
        # emitted after the startup projections so the Pool queue is free
        # for their bias evacuations; wv is first needed ~15 blocks in
        for i in range(6):
            nc.gpsimd.dma_start(w_t["v"][i][:], wv[ts(i, 128), :])
        for h in range(EH):
            nc.gpsimd.dma_start(wo_t[h][:], wo[ts(h, 128), :])

    # per-unit PE stream costs (ns) for schedule pacing
    QK_PE = 1278   # 6 matmuls F=512
    VH_PE = 320    # 6 matmuls F=128
    PV_PE = 852    # 4 matmuls F=512
    OP_PE = 960    # 3x F=512 + 3x F=256

    # ---- emission helpers ----------------------------------------------
    qT, kT = [None] * 3, [None] * 3
    v_t = qkv.tile([128, NT, 384], BF16, tag="v")
    # score psum ring: one 6-bank tile, slots S[:, k, :]; ring[0] counts
    # consumed slots.  When a sub-head's two 1024-halves land on adjacent
    # slots (no wrap), its two exps merge into ONE [128,2048] call with a
    # single accumulator read: 4 of every 6 sub-heads pair, saving ~24us
    # of ACT time over the kernel.
    S = spool.tile([128, 3, 1024], F32, tag="s", name="sring")
    ring = [0]
    outnT = []
    for h in range(EH):
        outnT.append(qkv.tile([128, N], BF16, tag=f"outnT{h}", name=f"outnT{h}"))

    def proj_qk_chunk(m, name, cc, tag="pv", cols=None, ts_eng=None):
        bias = bq_t if name == "q" else bk_t
        dst = qT if name == "q" else kT
        out_t = dst[m]
        c0, c1 = cols if cols else (cc * 512, cc * 512 + 512)
        w = c1 - c0
        ps = pvp.tile([128, 512], F32, tag=tag, name="psq")
        for kblk in range(6):
            nc.tensor.matmul(
                ps[:, 0:w],
                w_t[name][kblk][:, ts(m, 128)],
                xt[:, kblk, c0:c1],
                start=(kblk == 0),
                stop=(kblk == 5),
            )
        (ts_eng or nc.vector).tensor_scalar(
            out=out_t[:, c0:c1],
            in0=ps[:, 0:w],
            scalar1=bias[:, m : m + 1],
            scalar2=None,
            op0=OP.add,
        )

    def proj_qk_start(m):
        # the first attn block runs k-half-major: it needs kT[0] cc0/cc1
        # and qT[0] cc0 first (x cols 0:1024 + wk + wq land ~6us); kT[0]
        # cc2/cc3 (x cols 1024:2048, ~8.5us) are emitted by the block0
        # mid-callback between its two exp pairs so the first two score
        # matmuls sit at the head of the PE queue
        for name in ("q", "k"):
            dst = qT if name == "q" else kT
            dst[m] = qkv.tile(
                [128, N], BF16, tag=f"{name}T{m}", name=f"{name}T{m}"
            )
        proj_qk_chunk(m, "k", 0, tag="pv")
        proj_qk_chunk(m, "k", 1, tag="pvacc")
        # block0 only needs q cols 0:128: a short chunk gets its bias-TS
        # (the last exp1 dependency) done ~1.3us sooner
        proj_qk_chunk(m, "q", 0, tag="pv", cols=(0, 128))
        proj_qk_chunk(m, "q", 0, tag="pvacc", cols=(128, 512))
        return [
            (QK_PE, 1, lambda m=m, cc=cc: proj_qk_chunk(m, "q", cc))
            for cc in (1, 2, 3)
        ]

    def proj_k_late(m):
        proj_qk_chunk(m, "k", 2, tag="pvacc")
        proj_qk_chunk(m, "k", 3, tag="pv")

    def proj_qk_chunks(m):
        # deadline: head m's first score matmuls are emitted at group 4*m
        for name in ("q", "k"):
            dst = qT if name == "q" else kT
            dst[m] = qkv.tile(
                [128, N], BF16, tag=f"{name}T{m}", name=f"{name}T{m}"
            )
        return [
            (QK_PE, 4 * m, lambda m=m, name=name, cc=cc: proj_qk_chunk(m, name, cc))
            for name in ("q", "k")
            for cc in range(4)
        ]

    def proj_v_chunks(hh):
        # deadline: pv(hh, 0) chunks pop during group (hh, 1)
        return [
            (VH_PE, 4 * hh + 1, lambda tb=tb, hh=hh: proj_v(tb, hh))
            for tb in range(NT)
        ]

    def proj_v(tb, hh):
        ps = pvp.tile([128, 512], F32, tag="pv", name="psv")
        for kblk in range(6):
            nc.tensor.matmul(
                ps[:, 0:128],
                xt[:, kblk, ts(tb, 128)],
                w_t["v"][kblk][:, ts(hh, 128)],
                start=(kblk == 0),
                stop=(kblk == 5),
            )
        nc.vector.tensor_tensor(
            out=v_t[:, tb, ts(hh, 128)],
            in0=ps[:, 0:128],
            in1=bv_t[:, ts(hh, 128)],
            op=OP.add,
        )

    def attn_block(h, g, b, dT, lamp_g, d1_g, chunked=False, mid=None):
        """One 128-q-row block: scores+exp for both sub-heads, then the
        differential combination and its transpose.  sub1 (E2) runs first
        so d2s/rec overlap sub0's exps and lamp is ready right after the
        last accum lands.  mid (first block only) orders the exps by
        k-half and emits the late kT chunks between the two exp pairs."""
        t1 = g * 4 + b
        dacc = small.tile([128, 4], F32, tag="dacc", name="dacc")
        e_t = {}
        for sub in (1, 0):
            e_t[sub] = epool.tile([128, N], BF16, tag="E", name="e")

        def score_mms(sub, half, s):
            for c2 in range(2):
                cc = half * 2 + c2
                nc.tensor.matmul(
                    S[:, s, ts(c2, 512)],
                    qT[h][ts(sub, 64), ts(t1, 128)],
                    kT[h][ts(sub, 64), ts(cc, 512)],
                    start=True,
                    stop=True,
                )

        def exp_single(sub, half):
            s = ring[0] % 3
            ring[0] += 1
            score_mms(sub, half, s)
            slot = (1 - sub) * 2 + half
            nc.scalar.activation(
                e_t[sub][:, ts(half, 1024)],
                S[:, s, :],
                AF.Exp,
                accum_out=dacc[:, slot : slot + 1],
            )

        paired = {0: False, 1: False}
        if mid:
            # first block: k-half-major singles so the first exp pair can
            # run before x cols 1024:2048 land
            for ui, (sub, half) in enumerate([(0, 0), (1, 0), (0, 1), (1, 1)]):
                if ui == 2:
                    mid()
                exp_single(sub, half)
        else:
            for sub in (1, 0):
                p = ring[0] % 3
                if p <= 1:
                    # both halves on adjacent ring slots: one [128,2048]
                    # exp + one accumulator read
                    paired[sub] = True
                    for half in range(2):
                        score_mms(sub, half, p + half)
                    ring[0] += 2
                    sl = (1 - sub) * 2
                    nc.scalar.activation(
                        e_t[sub][:],
                        S[:, p : p + 2, :].rearrange("p a b -> p (a b)"),
                        AF.Exp,
                        accum_out=dacc[:, sl : sl + 1],
                    )
                else:
                    for half in range(2):
                        exp_single(sub, half)
                if sub == 1:
                    # 1/d2 computed while sub0's exps stream
                    rec = small.tile([128, 1], F32, tag="rec", name="rec")
                    if paired[1]:
                        nc.vector.reciprocal(rec[:], dacc[:, 0:1])
                    else:
                        d2s = small.tile([128, 1], F32, tag="d2s", name="d2s")
                        nc.vector.tensor_tensor(
                            out=d2s[:], in0=dacc[:, 0:1], in1=dacc[:, 1:2],
                            op=OP.add,
                        )
                        nc.vector.reciprocal(rec[:], d2s[:])
        if mid:
            rec = small.tile([128, 1], F32, tag="rec", name="rec")
            d2s = small.tile([128, 1], F32, tag="d2s", name="d2s")
            nc.vector.tensor_tensor(
                out=d2s[:], in0=dacc[:, 0:1], in1=dacc[:, 1:2], op=OP.add
            )
            nc.vector.reciprocal(rec[:], d2s[:])
        d1 = d1_g[:, b : b + 1]
        if paired[0]:
            nc.vector.tensor_copy(d1, dacc[:, 2:3])
        else:
            nc.vector.tensor_tensor(
                out=d1, in0=dacc[:, 2:3], in1=dacc[:, 3:4], op=OP.add
            )
        nc.vector.tensor_scalar(
            out=lamp_g[:, b : b + 1],
            in0=rec[:],
            scalar1=d1,
            scalar2=lam_t[:],
            op0=OP.mult,
            op1=OP.mult,
        )
        # diff = lamp*E2 - E1 in ONE pass (the sign flip is folded into Wo
        # on the host); engine alternates DVE/Pool to split the load
        diff = dpool.tile([128, N], BF16, tag="diff", name="diff")
        if chunked:
            # final group: 512-col chunks, alternating engines, so the
            # per-b PV pipeline starts ~1.5us earlier on the endgame
            # critical path; for the very last block the XBARs split over
            # the sync + (now idle) ACT queues
            for qc in range(4):
                nc.vector.scalar_tensor_tensor(
                    out=diff[:, ts(qc, 512)],
                    in0=e_t[1][:, ts(qc, 512)],
                    scalar=lamp_g[:, b : b + 1],
                    in1=e_t[0][:, ts(qc, 512)],
                    op0=OP.mult,
                    op1=OP.subtract,
                )
                xq = nc.scalar if (b == 3 and qc % 2 == 1) else nc.sync
                xq.dma_start(
                    dT[:, qc * 4 : qc * 4 + 4, ts(b, 128)],
                    diff[:, ts(qc, 512)],
                    transpose=True,
                )
        elif (g * 4 + b) % 2 == 0:
            # one-pass STT on DVE (TensorScalarPtr is not a Pool opcode)
            nc.vector.scalar_tensor_tensor(
                out=diff[:],
                in0=e_t[1][:],
                scalar=lamp_g[:, b : b + 1],
                in1=e_t[0][:],
                op0=OP.mult,
                op1=OP.subtract,
            )
            nc.sync.dma_start(dT[:, :, ts(b, 128)], diff[:], transpose=True)
        else:
            # alternate blocks: TS on DVE, subtract on Pool (SBUF-only)
            nc.vector.tensor_scalar(
                out=diff[:],
                in0=e_t[1][:],
                scalar1=lamp_g[:, b : b + 1],
                scalar2=None,
                op0=OP.mult,
            )
            nc.gpsimd.tensor_tensor(
                out=diff[:], in0=diff[:], in1=e_t[0][:], op=OP.subtract
            )
            # One batched XBAR transpose: dT[k%128, k//128, b*128+q] = diff[q, k]
            nc.sync.dma_start(dT[:, :, ts(b, 128)], diff[:], transpose=True)

    def pv_chunk(h, dT, pv, j0):
        for j in range(j0, j0 + 4):
            nc.tensor.matmul(
                pv[:],
                v_t[:, j, ts(h, 128)],
                dT[:, j, :],
                start=(j == 0),
                stop=(j == NT - 1),
            )

    def tail_pre(h, g, dT, d1_g, pv, st):
        """Group tail, part 1 (no ACT work): evacuate PV, transpose to
        [q, dv] layout, and compute the rms Ln argument on DVE."""
        outT = onp.tile([128, 512], BF16, tag="outT", name="outT")
        nc.vector.tensor_copy(outT[:], pv[:])
        nat_sb = onp.tile([128, 4, 128], BF16, tag="natsb", name="natsb")
        nc.sync.dma_start(nat_sb[:], outT[:], transpose=True)
        sq = onp.tile([128, 512], F32, tag="sq", name="sq")
        nc.vector.tensor_tensor(
            out=sq[:],
            in0=nat_sb[:].rearrange("p a b -> p (a b)"),
            in1=nat_sb[:].rearrange("p a b -> p (a b)"),
            op=OP.mult,
        )
        zg = small.tile([128, 4], F32, tag="zg", name="zg")
        for bb in range(4):
            nc.vector.tensor_reduce(
                out=zg[:, bb : bb + 1],
                in_=sq[:, ts(bb, 128)],
                axis=mybir.AxisListType.X,
                op=OP.add,
            )
        # rms arg = zg/128 + eps*d1^2, computed as (zg + 128*eps*d1*d1)/128
        # so a single batched Ln (const bias) replaces 4 per-b Ln calls
        epsd = small.tile([128, 4], F32, tag="epsd", name="epsd")
        nc.vector.scalar_tensor_tensor(
            out=epsd[:],
            in0=d1_g[:],
            scalar=EPS * 128.0,
            in1=d1_g[:],
            op0=OP.mult,
            op1=OP.mult,
        )
        targ = small.tile([128, 4], F32, tag="targ", name="targ")
        nc.vector.tensor_tensor(out=targ[:], in0=zg[:], in1=epsd[:], op=OP.add)
        st["nat"] = nat_sb
        st["targ"] = targ

    def tail_act(h, g, st):
        """Group tail, part 2 (the only non-exp ACT work): batched Ln/Exp
        rsqrt, scale, and transpose back.  Emitted a block later than
        tail_pre so the Ln never queue-blocks the next group's exps."""
        nat_sb, targ = st["nat"], st["targ"]
        lz = small.tile([128, 4], F32, tag="lz", name="lz")
        nc.scalar.activation(lz[:], targ[:], AF.Ln, scale=1.0 / 128.0)
        rg = small.tile([128, 4], F32, tag="rg", name="rg")
        nc.scalar.activation(rg[:], lz[:], AF.Exp, scale=-0.5)
        onn = onp.tile([128, 4, 128], BF16, tag="onn", name="onn")
        for bb in range(4):
            nc.vector.tensor_scalar(
                out=onn[:, bb, :],
                in0=nat_sb[:, bb, :],
                scalar1=rg[:, bb : bb + 1],
                scalar2=None,
                op0=OP.mult,
            )
        nc.sync.dma_start(
            outnT[h][:, ts(g, 512)].rearrange("p (a b) -> p a b", a=4),
            onn[:].rearrange("p a b -> p (a b)"),
            transpose=True,
        )

    def per_b_tail(h, g, b, dT, d1_g, pv):
        tb = g * 4 + b
        for j in range(NT):
            nc.tensor.matmul(
                pv[:, ts(b, 128)],
                v_t[:, j, ts(h, 128)],
                dT[:, j, ts(b, 128)],
                start=(j == 0),
                stop=(j == NT - 1),
            )
        outTb = onp.tile([128, 128], BF16, tag="outTb", name="outTb")
        nc.vector.tensor_copy(outTb[:], pv[:, ts(b, 128)])
        natb = onp.tile([128, 128], BF16, tag="natb", name="natb")
        # tails for b2/b3 are emitted after the final exps, so their
        # transposes can ride the then-idle ACT queue instead of sync
        xq = nc.scalar if b >= 2 else nc.sync
        xq.dma_start(natb[:], outTb[:], transpose=True)
        scrb = onp.tile([128, 128], F32, tag="scrb", name="scrb")
        nc.vector.tensor_tensor(out=scrb[:], in0=natb[:], in1=natb[:], op=OP.mult)
        zgb = small.tile([128, 1], F32, tag="zgb", name="zgb")
        nc.vector.tensor_reduce(
            out=zgb[:], in_=scrb[:], axis=mybir.AxisListType.X, op=OP.add
        )
        d1b = d1_g[:, b : b + 1]
        epsb = small.tile([128, 1], F32, tag="epsb", name="epsb")
        nc.vector.scalar_tensor_tensor(
            out=epsb[:], in0=d1b, scalar=EPS * 128.0, in1=d1b, op0=OP.mult, op1=OP.mult
        )
        targb = small.tile([128, 1], F32, tag="targb", name="targb")
        nc.vector.tensor_tensor(out=targb[:], in0=zgb[:], in1=epsb[:], op=OP.add)
        lzb = small.tile([128, 1], F32, tag="lzb", name="lzb")
        nc.scalar.activation(lzb[:], targb[:], AF.Ln, scale=1.0 / 128.0)
        rgb = small.tile([128, 1], F32, tag="rgb", name="rgb")
        nc.scalar.activation(rgb[:], lzb[:], AF.Exp, scale=-0.5)
        onnb = onp.tile([128, 128], BF16, tag="onnb", name="onnb")
        nc.vector.tensor_scalar(
            out=onnb[:], in0=natb[:], scalar1=rgb[:], scalar2=None, op0=OP.mult
        )
        xq.dma_start(outnT[h][:, ts(tb, 128)], onnb[:], transpose=True)

    def outproj_tb(tb):
        ya = pvp.tile([128, 512], F32, tag="pv", name="ya")
        for hh in range(EH):
            nc.tensor.matmul(
                ya[:],
                outnT[hh][:, ts(tb, 128)],
                wo_t[hh][:, 0:512],
                start=(hh == 0),
                stop=(hh == EH - 1),
            )
        yb = pvp.tile([128, 512], F32, tag="pv", name="yb")
        for hh in range(EH):
            nc.tensor.matmul(
                yb[:, 0:256],
                outnT[hh][:, ts(tb, 128)],
                wo_t[hh][:, 512:768],
                start=(hh == 0),
                stop=(hh == EH - 1),
            )
        # y rides the Pool DMA queue: keeps its traffic off the
        # XBAR-carrying sync queue during the endgame (Pool itself cannot
        # read PSUM, so the evacuation copies stay on DVE)
        yt = ysb.tile([128, D], F32, tag="y", name="yt")
        nc.vector.tensor_copy(yt[:, 0:512], ya[:])
        nc.gpsimd.dma_start(y[ts(tb, 128), 0:512], yt[:, 0:512])
        nc.vector.tensor_copy(yt[:, 512:768], yb[:, 0:256])
        nc.gpsimd.dma_start(y[ts(tb, 128), 512:768], yt[:, 512:768])

    def outproj_fast(tb):
        """Endgame out-projection: after the last exp the score psum ring
        is free, so ya/yb use rotating ring slots (no slot chain)."""
        s = ring[0] % 3
        ring[0] += 1
        for hh in range(EH):
            nc.tensor.matmul(
                S[:, s, 0:512],
                outnT[hh][:, ts(tb, 128)],
                wo_t[hh][:, 0:512],
                start=(hh == 0),
                stop=(hh == EH - 1),
            )
        for hh in range(EH):
            nc.tensor.matmul(
                S[:, s, 512:768],
                outnT[hh][:, ts(tb, 128)],
                wo_t[hh][:, 512:768],
                start=(hh == 0),
                stop=(hh == EH - 1),
            )
        yt = ysb.tile([128, D], F32, tag="y", name="ytf")
        nc.vector.tensor_copy(yt[:], S[:, s, 0:768])
        nc.gpsimd.dma_start(y[ts(tb, 128), :], yt[:])

    # ---- emission schedule ---------------------------------------------
    # Work deques of (pe_cost_ns, thunk):
    #   bg      — projection chunks (q rest, per-head v, later heads' q/k)
    #   pending — prior groups' PV chunks, tail_pre/tail_act, out-proj tiles
    # tail_act directly follows tail_pre: by the time ACT drains the 4.9us
    # of exps queued ahead of its Ln, the DVE rms-arg chain (~1.5us) has
    # long finished, so the Ln never head-of-line-blocks the exp stream.
    # Pops happen AFTER a block's score matmuls are enqueued, within a PE
    # budget, so ACT (exp, the critical engine) never waits on queued PE
    # work.  At b==3 only PE-free units pop: the next group's first score
    # matmuls must reach the PE queue with nothing in front of them.
    def pop_budget(q, budget):
        spent = 0
        while q:
            pe = q[0][0]
            if pe and spent + pe > budget:
                break
            spent += pe
            q.pop(0)[-1]()

    def pop_free(q):
        while q and q[0][0] == 0:
            q.pop(0)[-1]()

    def pop_overdue(q, gi):
        # correctness net: emission order defines the dependency graph, so
        # every projection chunk MUST be emitted before the first score
        # matmul that reads it; budgets are tuned so this never fires
        while q and q[0][1] <= gi:
            q.pop(0)[-1]()

    bg = (
        proj_qk_start(0)
        + proj_v_chunks(0)
        + proj_qk_chunks(1)
        + proj_v_chunks(1)
        + proj_qk_chunks(2)
        + proj_v_chunks(2)
    )
    late_w_dmas()
    # per-head (bg, bg@b2, pending, pending@b2) budgets: h0/h1 front-load
    # the next head's qk chunks (hard deadline: that head's first scores),
    # h2 drains pv+tails+outproj (batch ~10.6us/group); b2 tapers so
    # popped work never delays the next group's first score matmuls (b2's
    # taper still fits one 1278ns qk chunk so the deque keeps moving)
    BUDGETS = {
        0: (2200, 1400, 1400, 700),
        1: (2000, 1400, 2000, 1000),
        2: (1200, 600, 3400, 2200),
    }
    pending = []
    for h in range(EH):
        for g in range(4):
            first = h == 0 and g == 0
            last = h == EH - 1 and g == 3
            pop_overdue(bg, h * 4 + g)
            dT = dtp.tile([128, NT, 512], BF16, tag="dT", name="dT")
            lamp_g = small.tile([128, 4], F32, tag="lamp", name="lamp")
            d1_g = small.tile([128, 4], F32, tag="d1g", name="d1g")
            pv = pvp.tile([128, 512], F32, tag="pvacc", name="pv")
            for b in range(4):
                attn_block(
                    h, g, b, dT, lamp_g, d1_g,
                    chunked=last,
                    mid=(lambda: proj_k_late(0)) if first and b == 0 else None,
                )
                if last:
                    # final group: per-b pipeline, emitted 1 block late so
                    # the XBAR'd dT is ready when its PV matmuls dequeue
                    pop_budget(pending, 1800 if b == 2 else 2200)
                    if b >= 1:
                        per_b_tail(h, g, b - 1, dT, d1_g, pv)
                    continue
                bgb, bgb2, pb, pb2 = BUDGETS[h]
                if b == 3:
                    pop_free(pending)
                    if h == EH - 1:
                        pop_budget(pending, 1800)
                    continue
                if b == 2:
                    pop_budget(bg, bgb2)
                    pop_budget(pending, pb2)
                else:
                    pop_budget(bg, 4600 if first else bgb)
                    pop_budget(pending, pb)

            if last:
                # ops for tb12/13 are dep-free now and fill the dT(b3)
                # XBAR wait; op14 runs under b3's tail chain; leftover
                # pending ops (psum-slot chained) drain dead last
                outproj_fast(g * 4)
                outproj_fast(g * 4 + 1)
                per_b_tail(h, g, 3, dT, d1_g, pv)
                outproj_fast(g * 4 + 2)
                outproj_fast(g * 4 + 3)
                while pending:
                    pending.pop(0)[-1]()
                continue

            st = {}
            pending += [
                (PV_PE, lambda h=h, dT=dT, pv=pv, j0=j0: pv_chunk(h, dT, pv, j0))
                for j0 in (0, 4, 8, 12)
            ] + [
                (0, lambda h=h, g=g, dT=dT, d1_g=d1_g, pv=pv, st=st: tail_pre(
                    h, g, dT, d1_g, pv, st)),
                (0, lambda h=h, g=g, st=st: tail_act(h, g, st)),
            ]
            if h == EH - 1:
                # pseudo-cost 1800 (> real 960): the ya/yb psum slot is
                # held ~2.5us per op, so at most one op pops per block —
                # a second would head-of-line-block PE on the slot wait
                pending += [
                    (1800, lambda tb=tb: outproj_tb(tb))
                    for tb in range(g * 4, g * 4 + 4)
                ]
    for item in pending + bg:
        item[-1]()


def build_kernel():
    nc = bacc.Bacc("TRN2", target_bir_lowering=False, debug=False, num_devices=1)
    aps = [
        nc.dram_tensor("xT", [D, N], BF16, kind="ExternalInput").ap(),
        nc.dram_tensor("wq", [D, 384], BF16, kind="ExternalInput").ap(),
        nc.dram_tensor("wk", [D, 384], BF16, kind="ExternalInput").ap(),
        nc.dram_tensor("wv", [D, 384], BF16, kind="ExternalInput").ap(),
        nc.dram_tensor("wo", [384, D], BF16, kind="ExternalInput").ap(),
        nc.dram_tensor("bq", [128, 3], F32, kind="ExternalInput").ap(),
        nc.dram_tensor("bk", [128, 3], F32, kind="ExternalInput").ap(),
        nc.dram_tensor("bv128", [128, 384], F32, kind="ExternalInput").ap(),
        nc.dram_tensor("lam128", [128, 1], F32, kind="ExternalInput").ap(),
        nc.dram_tensor("y", [N, D], F32, kind="ExternalOutput").ap(),
    ]
    with tile.TileContext(nc, trace_sim=False) as tc:
        with ExitStack() as ctx:
            _body(ctx, tc, *aps)
    nc.compile()
    return nc


def make_in_maps(x, Wq, bqv, Wk, bkv, Wv, bvv, lambda_q1, lambda_k1,
                 lambda_q2, lambda_k2, norm_weight):
    scaling = HD ** -0.5
    lam1 = np.exp(np.sum(lambda_q1.astype(np.float64) * lambda_k1.astype(np.float64)))
    lam2 = np.exp(np.sum(lambda_q2.astype(np.float64) * lambda_k2.astype(np.float64)))
    lam = np.float32(lam1 - lam2 + LAMBDA_INIT)

    in_maps = []
    for c in range(NCORES):
        b = c // 2
        hs = 3 * (c % 2)
        cols = slice(128 * hs, 128 * (hs + 3))
        in_maps.append(
            {
                "xT": np.ascontiguousarray(x[b].T).astype(BF),
                "wq": np.ascontiguousarray(Wq[:, cols] * scaling).astype(BF),
                "wk": np.ascontiguousarray(Wk[:, cols]).astype(BF),
                "wv": np.ascontiguousarray(Wv[:, cols]).astype(BF),
                "wo": None,  # filled below (0.2*norm_weight folded in)
                "bq": np.ascontiguousarray(
                    (bqv[cols] * scaling).reshape(3, 128).T
                ).astype(np.float32),
                "bk": np.ascontiguousarray(bkv[cols].reshape(3, 128).T).astype(
                    np.float32
                ),
                "bv128": np.ascontiguousarray(
                    np.broadcast_to(bvv[cols], (128, 384))
                ).astype(np.float32),
                "lam128": np.full((128, 1), lam, np.float32),
            }
        )
    return in_maps, lam


def fold_wo(Wo, norm_weight):
    # norm_weight is a per-(2*HD)-lane diagonal scale right before Wo:
    # out_n @ (diag(0.2*wn) @ Wo) == (out_n*0.2*wn) @ Wo.  Folded on the
    # host, NEGATED: the kernel computes diff = lamp*E2 - E1 (one STT op)
    # which is -1x the reference's differential.
    return np.tile(-(1.0 - LAMBDA_INIT) * norm_weight, 6).reshape(768, 1) * Wo


_NC_CACHE = {}


def kernel(**inputs):
    x = np.asarray(inputs["x"], np.float32)
    Wq = np.asarray(inputs["Wq"], np.float32)
    bq = np.asarray(inputs["bq"], np.float32)
    Wk = np.asarray(inputs["Wk"], np.float32)
    bk = np.asarray(inputs["bk"], np.float32)
    Wv = np.asarray(inputs["Wv"], np.float32)
    bv = np.asarray(inputs["bv"], np.float32)
    Wo = np.asarray(inputs["Wo"], np.float32)
    bo = np.asarray(inputs["bo"], np.float32)
    norm_weight = np.asarray(inputs["norm_weight"], np.float32)

    in_maps, _lam = make_in_maps(
        x, Wq, bq, Wk, bk, Wv, bv,
        np.asarray(inputs["lambda_q1"], np.float32),
        np.asarray(inputs["lambda_k1"], np.float32),
        np.asarray(inputs["lambda_q2"], np.float32),
        np.asarray(inputs["lambda_k2"], np.float32),
        norm_weight,
    )
    wos = fold_wo(Wo, norm_weight)
    for c in range(NCORES):
        hs = 3 * (c % 2)
        cols = slice(128 * hs, 128 * (hs + 3))
        in_maps[c]["wo"] = np.ascontiguousarray(wos[cols, :]).astype(BF)

    if "nc" not in _NC_CACHE:
        _NC_CACHE["nc"] = build_kernel()
    nc = _NC_CACHE["nc"]

    res = bass_utils.run_bass_kernel_spmd(nc, in_maps, core_ids=list(range(NCORES)))

    out = np.empty((B, N, D), np.float32)
    for b in range(B):
        out[b] = res.results[2 * b]["y"] + res.results[2 * b + 1]["y"] + bo
    return out


# revision 55
# speedup vs baseline: 1.4225x; 1.4225x over previous
"""DiffAttention (nn_DiffAttention) — Trainium2 Bass kernel, 8 NeuronCores.

Sharding: 4 batches x 6 effective heads = 24 units -> core c gets batch
c//2 and effective heads [3*(c%2), 3*(c%2)+3).  Each core computes its
q/k/v projections (column-sliced), both softmax maps per eff-head, the
differential combination, head RMS-norm, and its partial output
projection; the host sums the two per-batch partials and adds bo.

All matmuls run in bf16 (fp32 PSUM accumulation).  Softmax is computed
without max-subtraction (scores are O(5) here) and without explicit
normalization: the softmax denominators d1, d2 enter through
lamp = lambda*d1/d2 and the eps-correction of the scale-invariant
RMS norm (rms(c*u) ~ c*rms(u)):

  E1 = exp(S1), E2 = exp(S2)         (ACT, accum_out -> d1, d2)
  diffn = lamp*E2 - E1               (DVE)
  u = diffn @ V                      (PE; ref out_pre = -(1/d1)*u)
  r = rsqrt(mean_dv(u^2) + eps*d1^2) (DVE + ACT ln/exp)
  out_n = u*r*(-0.2*wn)              (signs/scales cancel exactly)
  y += out_n @ Wo_slice              (PE)

ACT (exp: 192 calls of [128,1024], ~1.2us each) is the critical engine;
the schedule keeps its queue free of everything except exps and the
late-emitted, batched rms Ln/Exp tail calls.
"""

import os
import sys
from contextlib import ExitStack

import numpy as np

try:
    import concourse.bass as bass  # noqa: F401
except ImportError:
    for _p in ("/opt/trn_rl_repo", os.path.expanduser("~/trn_rl_repo")):
        if os.path.isdir(_p):
            sys.path.insert(0, _p)
            break
    import concourse.bass as bass  # noqa: F401

import ml_dtypes
import concourse.tile as tile
from concourse import bacc, bass_utils, mybir
from concourse.bass import ts

F32 = mybir.dt.float32
BF16 = mybir.dt.bfloat16
AF = mybir.ActivationFunctionType
OP = mybir.AluOpType

B = 4
N = 2048
D = 768
HD = 64
EH = 3  # eff heads per core
NT = N // 128
NCORES = 8
EPS = 1e-5
LAMBDA_INIT = 0.8
BF = ml_dtypes.bfloat16


def _body(ctx, tc, xT, wq, wk, wv, wo, bq, bk, bv128, lam128, y):
    nc = tc.nc

    const = ctx.enter_context(tc.tile_pool(name="const", bufs=1))
    wpool = ctx.enter_context(tc.tile_pool(name="wpool", bufs=1))
    xpool = ctx.enter_context(tc.tile_pool(name="xpool", bufs=1))
    qkv = ctx.enter_context(tc.tile_pool(name="qkv", bufs=1))
    # psum: s 3x2 banks + pv/proj 2x1 banks = 8 banks
    spool = ctx.enter_context(tc.tile_pool(name="spool", bufs=3, space="PSUM"))
    pvp = ctx.enter_context(tc.tile_pool(name="pvp", bufs=1, space="PSUM"))
    epool = ctx.enter_context(tc.tile_pool(name="epool", bufs=8))
    dpool = ctx.enter_context(tc.tile_pool(name="dpool", bufs=4))
    dtp = ctx.enter_context(tc.tile_pool(name="dtp", bufs=2))
    small = ctx.enter_context(tc.tile_pool(name="small", bufs=4))
    onp = ctx.enter_context(tc.tile_pool(name="onp", bufs=3))
    ysb = ctx.enter_context(tc.tile_pool(name="ysb", bufs=2))

    # ---- input DMAs -----------------------------------------------------
    # Startup is bound by per-queue serial DMA time (~0.39ns/B/partition),
    # so the exp1-critical pieces (x cols 0:1024, wk, wq) are split across
    # the sync and gpsimd queues so both finish ~6us; x cols 1024:2048
    # follow on both queues' tails.  The ACT queue issues no DMAs at all:
    # it must stay free for the exp stream.
    bq_t = const.tile([128, 3], F32)
    bk_t = const.tile([128, 3], F32)
    # warm the PE p-state ramp (full clock needs ~3us from first busy)
    # with zero matmuls so the first projection runs at 2.4GHz, not 0.65
    zpe = const.tile([128, 512], BF16)
    nc.vector.memset(zpe[:], 0)
    # warm the ACT exp/ln table while DMAs stream (first real exp would
    # otherwise pay the ~2.7us table load on the critical path)
    actwarm = const.tile([128, 1], F32)
    nc.scalar.activation(actwarm[:], zpe[:, 0:1], AF.Exp)
    nc.scalar.activation(actwarm[:], actwarm[:], AF.Ln)
    for _ in range(7):
        psz = pvp.tile([128, 512], F32, tag="pv", name="psz")
        nc.tensor.matmul(psz[:], zpe[:, 0:128], zpe[:], start=True, stop=True)
    xt = xpool.tile([128, 6, N], BF16, tag="xt", name="xt")

    def x_dma(eng, c0, c1):
        eng.dma_start(
            xt[:, :, c0:c1],
            xT[:, c0:c1].rearrange("(a p) c -> p a c", p=128),
        )

    # ACT's queue is idle until the first exp (~8us), so it carries the
    # tiny biases and one x quarter; nothing may queue on it after that
    x_dma(nc.sync, 0, 512)
    nc.scalar.dma_start(bk_t[:], bk)
    nc.scalar.dma_start(bq_t[:], bq)
    nc.scalar.dma_start(
        xt[:, :, 1024:1536],
        xT[:, 1024:1536].rearrange("(a p) c -> p a c", p=128),
    )
    w_t = {}
    for name, ap in (("k", wk), ("q", wq), ("v", wv)):
        tiles = []
        for i in range(6):
            t = wpool.tile([128, 384], BF16, tag=f"w{name}{i}", name=f"w{name}{i}")
            tiles.append(t)
        w_t[name] = tiles
    for i in range(6):
        nc.gpsimd.dma_start(w_t["k"][i][:], wk[ts(i, 128), :])
    x_dma(nc.sync, 512, 1024)
    for i in range(6):
        nc.gpsimd.dma_start(w_t["q"][i][:], wq[ts(i, 128), :])
    x_dma(nc.sync, 1536, 2048)
    lam_t = const.tile([128, 1], F32)
    nc.sync.dma_start(lam_t[:], lam128)
    bv_t = const.tile([128, 384], F32)
    nc.sync.dma_start(bv_t[:], bv128)
    wo_t = []
    for h in range(EH):
        t = wpool.tile([128, D], BF16, tag=f"wo{h}", name=f"wo{h}")
        wo_t.append(t)

    def late_w_dmas():
        # emitted after the startup projections so the Pool queue is free
        # for their bias evacuations; wv is first needed ~15 blocks in
        for i in range(6):
            nc.gpsimd.dma_start(w_t["v"][i][:], wv[ts(i, 128), :])
        for h in range(EH):
            nc.gpsimd.dma_start(wo_t[h][:], wo[ts(h, 128), :])

    # per-unit PE stream costs (ns) for schedule pacing
    QK_PE = 1278   # 6 matmuls F=512
    VH_PE = 320    # 6 matmuls F=128
    PV_PE = 852    # 4 matmuls F=512
    OP_PE = 960    # 3x F=512 + 3x F=256

    # ---- emission helpers ----------------------------------------------
    qT, kT = [None] * 3, [None] * 3
    v_t = qkv.tile([128, NT, 384], BF16, tag="v")
    outnT = []
    for h in range(EH):
        outnT.append(qkv.tile([128, N], BF16, tag=f"outnT{h}", name=f"outnT{h}"))

    def proj_qk_chunk(m, name, cc, tag="pv", cols=None, ts_eng=None):
        bias = bq_t if name == "q" else bk_t
        dst = qT if name == "q" else kT
        out_t = dst[m]
        c0, c1 = cols if cols else (cc * 512, cc * 512 + 512)
        w = c1 - c0
        ps = pvp.tile([128, 512], F32, tag=tag, name="psq")
        for kblk in range(6):
            nc.tensor.matmul(
                ps[:, 0:w],
                w_t[name][kblk][:, ts(m, 128)],
                xt[:, kblk, c0:c1],
                start=(kblk == 0),
                stop=(kblk == 5),
            )
        (ts_eng or nc.vector).tensor_scalar(
            out=out_t[:, c0:c1],
            in0=ps[:, 0:w],
            scalar1=bias[:, m : m + 1],
            scalar2=None,
            op0=OP.add,
        )

    def proj_qk_start(m):
        # the first attn block runs k-half-major: it needs kT[0] cc0/cc1
        # and qT[0] cc0 first (x cols 0:1024 + wk + wq land ~6us); kT[0]
        # cc2/cc3 (x cols 1024:2048, ~8.5us) are emitted by the block0
        # mid-callback between its two exp pairs so the first two score
        # matmuls sit at the head of the PE queue
        for name in ("q", "k"):
            dst = qT if name == "q" else kT
            dst[m] = qkv.tile(
                [128, N], BF16, tag=f"{name}T{m}", name=f"{name}T{m}"
            )
        proj_qk_chunk(m, "k", 0, tag="pv")
        proj_qk_chunk(m, "k", 1, tag="pvacc")
        # block0 only needs q cols 0:128: a short chunk gets its bias-TS
        # (the last exp1 dependency) done ~1.3us sooner
        proj_qk_chunk(m, "q", 0, tag="pv", cols=(0, 128))
        proj_qk_chunk(m, "q", 0, tag="pvacc", cols=(128, 512))
        return [
            (QK_PE, 1, lambda m=m, cc=cc: proj_qk_chunk(m, "q", cc))
            for cc in (1, 2, 3)
        ]

    def proj_k_late(m):
        proj_qk_chunk(m, "k", 2, tag="pvacc")
        proj_qk_chunk(m, "k", 3, tag="pv")

    def proj_qk_chunks(m):
        # deadline: head m's first score matmuls are emitted at group 4*m
        for name in ("q", "k"):
            dst = qT if name == "q" else kT
            dst[m] = qkv.tile(
                [128, N], BF16, tag=f"{name}T{m}", name=f"{name}T{m}"
            )
        return [
            (QK_PE, 4 * m, lambda m=m, name=name, cc=cc: proj_qk_chunk(m, name, cc))
            for name in ("q", "k")
            for cc in range(4)
        ]

    def proj_v_chunks(hh):
        # deadline: pv(hh, 0) chunks pop during group (hh, 1)
        return [
            (VH_PE, 4 * hh + 1, lambda tb=tb, hh=hh: proj_v(tb, hh))
            for tb in range(NT)
        ]

    def proj_v(tb, hh):
        ps = pvp.tile([128, 512], F32, tag="pv", name="psv")
        for kblk in range(6):
            nc.tensor.matmul(
                ps[:, 0:128],
                xt[:, kblk, ts(tb, 128)],
                w_t["v"][kblk][:, ts(hh, 128)],
                start=(kblk == 0),
                stop=(kblk == 5),
            )
        nc.vector.tensor_tensor(
            out=v_t[:, tb, ts(hh, 128)],
            in0=ps[:, 0:128],
            in1=bv_t[:, ts(hh, 128)],
            op=OP.add,
        )

    def attn_block(h, g, b, dT, lamp_g, d1_g, chunked=False, mid=None):
        """One 128-q-row block: scores+exp for both sub-heads, then the
        differential combination and its transpose.  sub1 (E2) runs first
        so d2s/rec overlap sub0's exps and lamp is ready right after the
        last accum lands.  mid (first block only) orders the exps by
        k-half and emits the late kT chunks between the two exp pairs."""
        t1 = g * 4 + b
        dacc = small.tile([128, 4], F32, tag="dacc", name="dacc")
        e_t = {}
        for sub in (1, 0):
            e_t[sub] = epool.tile([128, N], BF16, tag="E", name="e")

        def exp_single(sub, half):
            ps = spool.tile([128, 1024], F32, tag="s", name="ps")
            for c2 in range(2):
                cc = half * 2 + c2
                nc.tensor.matmul(
                    ps[:, ts(c2, 512)],
                    qT[h][ts(sub, 64), ts(t1, 128)],
                    kT[h][ts(sub, 64), ts(cc, 512)],
                    start=True,
                    stop=True,
                )
            slot = (1 - sub) * 2 + half
            nc.scalar.activation(
                e_t[sub][:, ts(half, 1024)],
                ps[:],
                AF.Exp,
                accum_out=dacc[:, slot : slot + 1],
            )

        units = (
            [(0, 0), (1, 0), (0, 1), (1, 1)]
            if mid
            else [(1, 0), (1, 1), (0, 0), (0, 1)]
        )
        for ui, (sub, half) in enumerate(units):
            if mid and ui == 2:
                mid()
            exp_single(sub, half)
            if (sub, half) == (1, 1):
                # d2 and 1/d2 computed while sub0's exps stream
                d2s = small.tile([128, 1], F32, tag="d2s", name="d2s")
                nc.vector.tensor_tensor(
                    out=d2s[:], in0=dacc[:, 0:1], in1=dacc[:, 1:2], op=OP.add
                )
                rec = small.tile([128, 1], F32, tag="rec", name="rec")
                nc.vector.reciprocal(rec[:], d2s[:])
        d1 = d1_g[:, b : b + 1]
        nc.vector.tensor_tensor(
            out=d1, in0=dacc[:, 2:3], in1=dacc[:, 3:4], op=OP.add
        )
        nc.vector.tensor_scalar(
            out=lamp_g[:, b : b + 1],
            in0=rec[:],
            scalar1=d1,
            scalar2=lam_t[:],
            op0=OP.mult,
            op1=OP.mult,
        )
        # diff = lamp*E2 - E1 in ONE pass (the sign flip is folded into Wo
        # on the host); engine alternates DVE/Pool to split the load
        diff = dpool.tile([128, N], BF16, tag="diff", name="diff")
        if chunked:
            # final group: 512-col chunks, alternating engines, so the
            # per-b PV pipeline starts ~1.5us earlier on the endgame
            # critical path; for the very last block the XBARs split over
            # the sync + (now idle) ACT queues
            for qc in range(4):
                nc.vector.scalar_tensor_tensor(
                    out=diff[:, ts(qc, 512)],
                    in0=e_t[1][:, ts(qc, 512)],
                    scalar=lamp_g[:, b : b + 1],
                    in1=e_t[0][:, ts(qc, 512)],
                    op0=OP.mult,
                    op1=OP.subtract,
                )
                xq = nc.scalar if (b == 3 and qc % 2 == 1) else nc.sync
                xq.dma_start(
                    dT[:, qc * 4 : qc * 4 + 4, ts(b, 128)],
                    diff[:, ts(qc, 512)],
                    transpose=True,
                )
        elif (g * 4 + b) % 2 == 0:
            # one-pass STT on DVE (TensorScalarPtr is not a Pool opcode)
            nc.vector.scalar_tensor_tensor(
                out=diff[:],
                in0=e_t[1][:],
                scalar=lamp_g[:, b : b + 1],
                in1=e_t[0][:],
                op0=OP.mult,
                op1=OP.subtract,
            )
            nc.sync.dma_start(dT[:, :, ts(b, 128)], diff[:], transpose=True)
        else:
            # alternate blocks: TS on DVE, subtract on Pool (SBUF-only)
            nc.vector.tensor_scalar(
                out=diff[:],
                in0=e_t[1][:],
                scalar1=lamp_g[:, b : b + 1],
                scalar2=None,
                op0=OP.mult,
            )
            nc.gpsimd.tensor_tensor(
                out=diff[:], in0=diff[:], in1=e_t[0][:], op=OP.subtract
            )
            # One batched XBAR transpose: dT[k%128, k//128, b*128+q] = diff[q, k]
            nc.sync.dma_start(dT[:, :, ts(b, 128)], diff[:], transpose=True)

    def pv_chunk(h, dT, pv, j0):
        for j in range(j0, j0 + 4):
            nc.tensor.matmul(
                pv[:],
                v_t[:, j, ts(h, 128)],
                dT[:, j, :],
                start=(j == 0),
                stop=(j == NT - 1),
            )

    def tail_pre(h, g, dT, d1_g, pv, st):
        """Group tail, part 1 (no ACT work): evacuate PV, transpose to
        [q, dv] layout, and compute the rms Ln argument on DVE."""
        outT = onp.tile([128, 512], BF16, tag="outT", name="outT")
        nc.vector.tensor_copy(outT[:], pv[:])
        nat_sb = onp.tile([128, 4, 128], BF16, tag="natsb", name="natsb")
        nc.sync.dma_start(nat_sb[:], outT[:], transpose=True)
        sq = onp.tile([128, 512], F32, tag="sq", name="sq")
        nc.vector.tensor_tensor(
            out=sq[:],
            in0=nat_sb[:].rearrange("p a b -> p (a b)"),
            in1=nat_sb[:].rearrange("p a b -> p (a b)"),
            op=OP.mult,
        )
        zg = small.tile([128, 4], F32, tag="zg", name="zg")
        for bb in range(4):
            nc.vector.tensor_reduce(
                out=zg[:, bb : bb + 1],
                in_=sq[:, ts(bb, 128)],
                axis=mybir.AxisListType.X,
                op=OP.add,
            )
        # rms arg = zg/128 + eps*d1^2, computed as (zg + 128*eps*d1*d1)/128
        # so a single batched Ln (const bias) replaces 4 per-b Ln calls
        epsd = small.tile([128, 4], F32, tag="epsd", name="epsd")
        nc.vector.scalar_tensor_tensor(
            out=epsd[:],
            in0=d1_g[:],
            scalar=EPS * 128.0,
            in1=d1_g[:],
            op0=OP.mult,
            op1=OP.mult,
        )
        targ = small.tile([128, 4], F32, tag="targ", name="targ")
        nc.vector.tensor_tensor(out=targ[:], in0=zg[:], in1=epsd[:], op=OP.add)
        st["nat"] = nat_sb
        st["targ"] = targ

    def tail_act(h, g, st):
        """Group tail, part 2 (the only non-exp ACT work): batched Ln/Exp
        rsqrt, scale, and transpose back.  Emitted a block later than
        tail_pre so the Ln never queue-blocks the next group's exps."""
        nat_sb, targ = st["nat"], st["targ"]
        lz = small.tile([128, 4], F32, tag="lz", name="lz")
        nc.scalar.activation(lz[:], targ[:], AF.Ln, scale=1.0 / 128.0)
        rg = small.tile([128, 4], F32, tag="rg", name="rg")
        nc.scalar.activation(rg[:], lz[:], AF.Exp, scale=-0.5)
        onn = onp.tile([128, 4, 128], BF16, tag="onn", name="onn")
        for bb in range(4):
            nc.vector.tensor_scalar(
                out=onn[:, bb, :],
                in0=nat_sb[:, bb, :],
                scalar1=rg[:, bb : bb + 1],
                scalar2=None,
                op0=OP.mult,
            )
        nc.sync.dma_start(
            outnT[h][:, ts(g, 512)].rearrange("p (a b) -> p a b", a=4),
            onn[:].rearrange("p a b -> p (a b)"),
            transpose=True,
        )

    def per_b_tail(h, g, b, dT, d1_g, pv):
        tb = g * 4 + b
        for j in range(NT):
            nc.tensor.matmul(
                pv[:, ts(b, 128)],
                v_t[:, j, ts(h, 128)],
                dT[:, j, ts(b, 128)],
                start=(j == 0),
                stop=(j == NT - 1),
            )
        outTb = onp.tile([128, 128], BF16, tag="outTb", name="outTb")
        nc.vector.tensor_copy(outTb[:], pv[:, ts(b, 128)])
        natb = onp.tile([128, 128], BF16, tag="natb", name="natb")
        # tails for b2/b3 are emitted after the final exps, so their
        # transposes can ride the then-idle ACT queue instead of sync
        xq = nc.scalar if b >= 2 else nc.sync
        xq.dma_start(natb[:], outTb[:], transpose=True)
        scrb = onp.tile([128, 128], F32, tag="scrb", name="scrb")
        nc.vector.tensor_tensor(out=scrb[:], in0=natb[:], in1=natb[:], op=OP.mult)
        zgb = small.tile([128, 1], F32, tag="zgb", name="zgb")
        nc.vector.tensor_reduce(
            out=zgb[:], in_=scrb[:], axis=mybir.AxisListType.X, op=OP.add
        )
        d1b = d1_g[:, b : b + 1]
        epsb = small.tile([128, 1], F32, tag="epsb", name="epsb")
        nc.vector.scalar_tensor_tensor(
            out=epsb[:], in0=d1b, scalar=EPS * 128.0, in1=d1b, op0=OP.mult, op1=OP.mult
        )
        targb = small.tile([128, 1], F32, tag="targb", name="targb")
        nc.vector.tensor_tensor(out=targb[:], in0=zgb[:], in1=epsb[:], op=OP.add)
        lzb = small.tile([128, 1], F32, tag="lzb", name="lzb")
        nc.scalar.activation(lzb[:], targb[:], AF.Ln, scale=1.0 / 128.0)
        rgb = small.tile([128, 1], F32, tag="rgb", name="rgb")
        nc.scalar.activation(rgb[:], lzb[:], AF.Exp, scale=-0.5)
        onnb = onp.tile([128, 128], BF16, tag="onnb", name="onnb")
        nc.vector.tensor_scalar(
            out=onnb[:], in0=natb[:], scalar1=rgb[:], scalar2=None, op0=OP.mult
        )
        xq.dma_start(outnT[h][:, ts(tb, 128)], onnb[:], transpose=True)

    def outproj_tb(tb):
        ya = pvp.tile([128, 512], F32, tag="pv", name="ya")
        for hh in range(EH):
            nc.tensor.matmul(
                ya[:],
                outnT[hh][:, ts(tb, 128)],
                wo_t[hh][:, 0:512],
                start=(hh == 0),
                stop=(hh == EH - 1),
            )
        yb = pvp.tile([128, 512], F32, tag="pv", name="yb")
        for hh in range(EH):
            nc.tensor.matmul(
                yb[:, 0:256],
                outnT[hh][:, ts(tb, 128)],
                wo_t[hh][:, 512:768],
                start=(hh == 0),
                stop=(hh == EH - 1),
            )
        # y rides the Pool DMA queue: keeps its traffic off the
        # XBAR-carrying sync queue during the endgame (Pool itself cannot
        # read PSUM, so the evacuation copies stay on DVE)
        yt = ysb.tile([128, D], F32, tag="y", name="yt")
        nc.vector.tensor_copy(yt[:, 0:512], ya[:])
        nc.gpsimd.dma_start(y[ts(tb, 128), 0:512], yt[:, 0:512])
        nc.vector.tensor_copy(yt[:, 512:768], yb[:, 0:256])
        nc.gpsimd.dma_start(y[ts(tb, 128), 512:768], yt[:, 512:768])

    def outproj_fast(tb):
        """Endgame out-projection: after the last exp the score psum ring
        is free, so ya/yb go to one 2-bank spool tile (no slot chain)."""
        t = spool.tile([128, 1024], F32, tag="s", name="opf")
        for hh in range(EH):
            nc.tensor.matmul(
                t[:, 0:512],
                outnT[hh][:, ts(tb, 128)],
                wo_t[hh][:, 0:512],
                start=(hh == 0),
                stop=(hh == EH - 1),
            )
        for hh in range(EH):
            nc.tensor.matmul(
                t[:, 512:768],
                outnT[hh][:, ts(tb, 128)],
                wo_t[hh][:, 512:768],
                start=(hh == 0),
                stop=(hh == EH - 1),
            )
        yt = ysb.tile([128, D], F32, tag="y", name="ytf")
        nc.vector.tensor_copy(yt[:], t[:, 0:768])
        nc.gpsimd.dma_start(y[ts(tb, 128), :], yt[:])

    # ---- emission schedule ---------------------------------------------
    # Work deques of (pe_cost_ns, thunk):
    #   bg      — projection chunks (q rest, per-head v, later heads' q/k)
    #   pending — prior groups' PV chunks, tail_pre/tail_act, out-proj tiles
    # tail_act directly follows tail_pre: by the time ACT drains the 4.9us
    # of exps queued ahead of its Ln, the DVE rms-arg chain (~1.5us) has
    # long finished, so the Ln never head-of-line-blocks the exp stream.
    # Pops happen AFTER a block's score matmuls are enqueued, within a PE
    # budget, so ACT (exp, the critical engine) never waits on queued PE
    # work.  At b==3 only PE-free units pop: the next group's first score
    # matmuls must reach the PE queue with nothing in front of them.
    def pop_budget(q, budget):
        spent = 0
        while q:
            pe = q[0][0]
            if pe and spent + pe > budget:
                break
            spent += pe
            q.pop(0)[-1]()

    def pop_free(q):
        while q and q[0][0] == 0:
            q.pop(0)[-1]()

    def pop_overdue(q, gi):
        # correctness net: emission order defines the dependency graph, so
        # every projection chunk MUST be emitted before the first score
        # matmul that reads it; budgets are tuned so this never fires
        while q and q[0][1] <= gi:
            q.pop(0)[-1]()

    bg = (
        proj_qk_start(0)
        + proj_v_chunks(0)
        + proj_qk_chunks(1)
        + proj_v_chunks(1)
        + proj_qk_chunks(2)
        + proj_v_chunks(2)
    )
    late_w_dmas()
    # per-head (bg, bg@b2, pending, pending@b2) budgets: h0/h1 front-load
    # the next head's qk chunks (hard deadline: that head's first scores),
    # h2 drains pv+tails+outproj (batch ~10.6us/group); b2 tapers so
    # popped work never delays the next group's first score matmuls (b2's
    # taper still fits one 1278ns qk chunk so the deque keeps moving)
    BUDGETS = {
        0: (2200, 1400, 1400, 700),
        1: (2000, 1400, 2000, 1000),
        2: (1200, 600, 3400, 2200),
    }
    pending = []
    for h in range(EH):
        for g in range(4):
            first = h == 0 and g == 0
            last = h == EH - 1 and g == 3
            pop_overdue(bg, h * 4 + g)
            dT = dtp.tile([128, NT, 512], BF16, tag="dT", name="dT")
            lamp_g = small.tile([128, 4], F32, tag="lamp", name="lamp")
            d1_g = small.tile([128, 4], F32, tag="d1g", name="d1g")
            pv = pvp.tile([128, 512], F32, tag="pvacc", name="pv")
            for b in range(4):
                attn_block(
                    h, g, b, dT, lamp_g, d1_g,
                    chunked=last,
                    mid=(lambda: proj_k_late(0)) if first and b == 0 else None,
                )
                if last:
                    # final group: per-b pipeline, emitted 1 block late so
                    # the XBAR'd dT is ready when its PV matmuls dequeue
                    pop_budget(pending, 1800 if b == 2 else 2200)
                    if b >= 1:
                        per_b_tail(h, g, b - 1, dT, d1_g, pv)
                    continue
                bgb, bgb2, pb, pb2 = BUDGETS[h]
                if b == 3:
                    pop_free(pending)
                    if h == EH - 1:
                        pop_budget(pending, 1800)
                    continue
                if b == 2:
                    pop_budget(bg, bgb2)
                    pop_budget(pending, pb2)
                else:
                    pop_budget(bg, 4600 if first else bgb)
                    pop_budget(pending, pb)

            if last:
                # ops for tb12/13 are dep-free now and fill the dT(b3)
                # XBAR wait; op14 runs under b3's tail chain; leftover
                # pending ops (psum-slot chained) drain dead last
                outproj_fast(g * 4)
                outproj_fast(g * 4 + 1)
                per_b_tail(h, g, 3, dT, d1_g, pv)
                outproj_fast(g * 4 + 2)
                outproj_fast(g * 4 + 3)
                while pending:
                    pending.pop(0)[-1]()
                continue

            st = {}
            pending += [
                (PV_PE, lambda h=h, dT=dT, pv=pv, j0=j0: pv_chunk(h, dT, pv, j0))
                for j0 in (0, 4, 8, 12)
            ] + [
                (0, lambda h=h, g=g, dT=dT, d1_g=d1_g, pv=pv, st=st: tail_pre(
                    h, g, dT, d1_g, pv, st)),
                (0, lambda h=h, g=g, st=st: tail_act(h, g, st)),
            ]
            if h == EH - 1:
                # pseudo-cost 1800 (> real 960): the ya/yb psum slot is
                # held ~2.5us per op, so at most one op pops per block —
                # a second would head-of-line-block PE on the slot wait
                pending += [
                    (1800, lambda tb=tb: outproj_tb(tb))
                    for tb in range(g * 4, g * 4 + 4)
                ]
    for item in pending + bg:
        item[-1]()


def build_kernel():
    nc = bacc.Bacc("TRN2", target_bir_lowering=False, debug=False, num_devices=1)
    aps = [
        nc.dram_tensor("xT", [D, N], BF16, kind="ExternalInput").ap(),
        nc.dram_tensor("wq", [D, 384], BF16, kind="ExternalInput").ap(),
        nc.dram_tensor("wk", [D, 384], BF16, kind="ExternalInput").ap(),
        nc.dram_tensor("wv", [D, 384], BF16, kind="ExternalInput").ap(),
        nc.dram_tensor("wo", [384, D], BF16, kind="ExternalInput").ap(),
        nc.dram_tensor("bq", [128, 3], F32, kind="ExternalInput").ap(),
        nc.dram_tensor("bk", [128, 3], F32, kind="ExternalInput").ap(),
        nc.dram_tensor("bv128", [128, 384], F32, kind="ExternalInput").ap(),
        nc.dram_tensor("lam128", [128, 1], F32, kind="ExternalInput").ap(),
        nc.dram_tensor("y", [N, D], F32, kind="ExternalOutput").ap(),
    ]
    with tile.TileContext(nc, trace_sim=False) as tc:
        with ExitStack() as ctx:
            _body(ctx, tc, *aps)
    nc.compile()
    return nc


def make_in_maps(x, Wq, bqv, Wk, bkv, Wv, bvv, lambda_q1, lambda_k1,
                 lambda_q2, lambda_k2, norm_weight):
    scaling = HD ** -0.5
    lam1 = np.exp(np.sum(lambda_q1.astype(np.float64) * lambda_k1.astype(np.float64)))
    lam2 = np.exp(np.sum(lambda_q2.astype(np.float64) * lambda_k2.astype(np.float64)))
    lam = np.float32(lam1 - lam2 + LAMBDA_INIT)

    in_maps = []
    for c in range(NCORES):
        b = c // 2
        hs = 3 * (c % 2)
        cols = slice(128 * hs, 128 * (hs + 3))
        in_maps.append(
            {
                "xT": np.ascontiguousarray(x[b].T).astype(BF),
                "wq": np.ascontiguousarray(Wq[:, cols] * scaling).astype(BF),
                "wk": np.ascontiguousarray(Wk[:, cols]).astype(BF),
                "wv": np.ascontiguousarray(Wv[:, cols]).astype(BF),
                "wo": None,  # filled below (0.2*norm_weight folded in)
                "bq": np.ascontiguousarray(
                    (bqv[cols] * scaling).reshape(3, 128).T
                ).astype(np.float32),
                "bk": np.ascontiguousarray(bkv[cols].reshape(3, 128).T).astype(
                    np.float32
                ),
                "bv128": np.ascontiguousarray(
                    np.broadcast_to(bvv[cols], (128, 384))
                ).astype(np.float32),
                "lam128": np.full((128, 1), lam, np.float32),
            }
        )
    return in_maps, lam


def fold_wo(Wo, norm_weight):
    # norm_weight is a per-(2*HD)-lane diagonal scale right before Wo:
    # out_n @ (diag(0.2*wn) @ Wo) == (out_n*0.2*wn) @ Wo.  Folded on the
    # host, NEGATED: the kernel computes diff = lamp*E2 - E1 (one STT op)
    # which is -1x the reference's differential.
    return np.tile(-(1.0 - LAMBDA_INIT) * norm_weight, 6).reshape(768, 1) * Wo


_NC_CACHE = {}


def kernel(**inputs):
    x = np.asarray(inputs["x"], np.float32)
    Wq = np.asarray(inputs["Wq"], np.float32)
    bq = np.asarray(inputs["bq"], np.float32)
    Wk = np.asarray(inputs["Wk"], np.float32)
    bk = np.asarray(inputs["bk"], np.float32)
    Wv = np.asarray(inputs["Wv"], np.float32)
    bv = np.asarray(inputs["bv"], np.float32)
    Wo = np.asarray(inputs["Wo"], np.float32)
    bo = np.asarray(inputs["bo"], np.float32)
    norm_weight = np.asarray(inputs["norm_weight"], np.float32)

    in_maps, _lam = make_in_maps(
        x, Wq, bq, Wk, bk, Wv, bv,
        np.asarray(inputs["lambda_q1"], np.float32),
        np.asarray(inputs["lambda_k1"], np.float32),
        np.asarray(inputs["lambda_q2"], np.float32),
        np.asarray(inputs["lambda_k2"], np.float32),
        norm_weight,
    )
    wos = fold_wo(Wo, norm_weight)
    for c in range(NCORES):
        hs = 3 * (c % 2)
        cols = slice(128 * hs, 128 * (hs + 3))
        in_maps[c]["wo"] = np.ascontiguousarray(wos[cols, :]).astype(BF)

    if "nc" not in _NC_CACHE:
        _NC_CACHE["nc"] = build_kernel()
    nc = _NC_CACHE["nc"]

    res = bass_utils.run_bass_kernel_spmd(nc, in_maps, core_ids=list(range(NCORES)))

    out = np.empty((B, N, D), np.float32)
    for b in range(B):
        out[b] = res.results[2 * b]["y"] + res.results[2 * b + 1]["y"] + bo
    return out


# revision 57
# speedup vs baseline: 1.6718x; 1.1753x over previous
"""DiffAttention (nn_DiffAttention) — Trainium2 Bass kernel, 8 NeuronCores.

Sharding: 4 batches x 6 effective heads = 24 units -> core c gets batch
c//2 and effective heads [3*(c%2), 3*(c%2)+3).  Each core computes its
q/k/v projections (column-sliced), both softmax maps per eff-head, the
differential combination, head RMS-norm, and its partial output
projection; the host sums the two per-batch partials and adds bo.

All matmuls run in bf16 (fp32 PSUM accumulation).  Softmax is computed
without max-subtraction (scores are O(5) here) and without explicit
normalization: the softmax denominators d1, d2 enter through
lamp = lambda*d1/d2 and the eps-correction of the scale-invariant
RMS norm (rms(c*u) ~ c*rms(u)):

  E1 = exp(S1), E2 = exp(S2)         (ACT, accum_out -> d1, d2)
  diffn = lamp*E2 - E1               (DVE)
  u = diffn @ V                      (PE; ref out_pre = -(1/d1)*u)
  r = rsqrt(mean_dv(u^2) + eps*d1^2) (DVE + ACT ln/exp)
  out_n = u*r*(-0.2*wn)              (signs/scales cancel exactly)
  y += out_n @ Wo_slice              (PE)

ACT (exp: 192 calls of [128,1024], ~1.2us each) is the critical engine;
the schedule keeps its queue free of everything except exps and the
late-emitted, batched rms Ln/Exp tail calls.
"""

import os
import sys
from contextlib import ExitStack

import numpy as np

try:
    import concourse.bass as bass  # noqa: F401
except ImportError:
    for _p in ("/opt/trn_rl_repo", os.path.expanduser("~/trn_rl_repo")):
        if os.path.isdir(_p):
            sys.path.insert(0, _p)
            break
    import concourse.bass as bass  # noqa: F401

import ml_dtypes
import concourse.tile as tile
from concourse import bacc, bass_utils, mybir
from concourse.bass import ts

F32 = mybir.dt.float32
BF16 = mybir.dt.bfloat16
AF = mybir.ActivationFunctionType
OP = mybir.AluOpType

B = 4
N = 2048
D = 768
HD = 64
EH = 3  # eff heads per core
NT = N // 128
NCORES = 8
EPS = 1e-5
LAMBDA_INIT = 0.8
BF = ml_dtypes.bfloat16


def _body(ctx, tc, xT, wq, wk, wv, wo, bq, bk, bv128, lam128, y):
    nc = tc.nc

    const = ctx.enter_context(tc.tile_pool(name="const", bufs=1))
    wpool = ctx.enter_context(tc.tile_pool(name="wpool", bufs=1))
    xpool = ctx.enter_context(tc.tile_pool(name="xpool", bufs=1))
    qkv = ctx.enter_context(tc.tile_pool(name="qkv", bufs=1))
    # psum: s 3x2 banks + pv/proj 2x1 banks = 8 banks
    spool = ctx.enter_context(tc.tile_pool(name="spool", bufs=3, space="PSUM"))
    pvp = ctx.enter_context(tc.tile_pool(name="pvp", bufs=1, space="PSUM"))
    epool = ctx.enter_context(tc.tile_pool(name="epool", bufs=8))
    dpool = ctx.enter_context(tc.tile_pool(name="dpool", bufs=4))
    dtp = ctx.enter_context(tc.tile_pool(name="dtp", bufs=2))
    small = ctx.enter_context(tc.tile_pool(name="small", bufs=4))
    onp = ctx.enter_context(tc.tile_pool(name="onp", bufs=3))
    ysb = ctx.enter_context(tc.tile_pool(name="ysb", bufs=2))

    # ---- input DMAs -----------------------------------------------------
    # Startup is bound by per-queue serial DMA time (~0.39ns/B/partition),
    # so the exp1-critical pieces (x cols 0:1024, wk, wq) are split across
    # the sync and gpsimd queues so both finish ~6us; x cols 1024:2048
    # follow on both queues' tails.  The ACT queue issues no DMAs at all:
    # it must stay free for the exp stream.
    bq_t = const.tile([128, 3], F32)
    bk_t = const.tile([128, 3], F32)
    # warm the PE p-state ramp (full clock needs ~3us from first busy)
    # with zero matmuls so the first projection runs at 2.4GHz, not 0.65
    zpe = const.tile([128, 512], BF16)
    nc.vector.memset(zpe[:], 0)
    # warm the ACT exp/ln table while DMAs stream (first real exp would
    # otherwise pay the ~2.7us table load on the critical path)
    actwarm = const.tile([128, 1], F32)
    nc.scalar.activation(actwarm[:], zpe[:, 0:1], AF.Exp)
    nc.scalar.activation(actwarm[:], actwarm[:], AF.Ln)
    for _ in range(7):
        psz = pvp.tile([128, 512], F32, tag="pv", name="psz")
        nc.tensor.matmul(psz[:], zpe[:, 0:128], zpe[:], start=True, stop=True)
    xt = xpool.tile([128, 6, N], BF16, tag="xt", name="xt")

    def x_dma(eng, c0, c1):
        eng.dma_start(
            xt[:, :, c0:c1],
            xT[:, c0:c1].rearrange("(a p) c -> p a c", p=128),
        )

    # ACT's queue is idle until the first exp (~8us), so it carries the
    # tiny biases and one x quarter; nothing may queue on it after that
    x_dma(nc.sync, 0, 512)
    nc.scalar.dma_start(bk_t[:], bk)
    nc.scalar.dma_start(bq_t[:], bq)
    nc.scalar.dma_start(
        xt[:, :, 1024:1536],
        xT[:, 1024:1536].rearrange("(a p) c -> p a c", p=128),
    )
    w_t = {}
    for name, ap in (("k", wk), ("q", wq), ("v", wv)):
        tiles = []
        for i in range(6):
            t = wpool.tile([128, 384], BF16, tag=f"w{name}{i}", name=f"w{name}{i}")
            tiles.append(t)
        w_t[name] = tiles
    for i in range(6):
        nc.gpsimd.dma_start(w_t["k"][i][:], wk[ts(i, 128), :])
    x_dma(nc.sync, 512, 1024)
    for i in range(6):
        nc.gpsimd.dma_start(w_t["q"][i][:], wq[ts(i, 128), :])
    x_dma(nc.sync, 1536, 2048)
    lam_t = const.tile([128, 1], F32)
    nc.sync.dma_start(lam_t[:], lam128)
    bv_t = const.tile([128, 384], F32)
    nc.sync.dma_start(bv_t[:], bv128)
    wo_t = []
    for h in range(EH):
        t = wpool.tile([128, D], BF16, tag=f"wo{h}", name=f"wo{h}")
        wo_t.append(t)

    def late_w_dmas():
        # emitted after the startup projections so the Pool queue is free
        # for their bias evacuations; wv is first needed ~15 blocks in
        for i in range(6):
            nc.gpsimd.dma_start(w_t["v"][i][:], wv[ts(i, 128), :])
        for h in range(EH):
            nc.gpsimd.dma_start(wo_t[h][:], wo[ts(h, 128), :])

    # per-unit PE stream costs (ns) for schedule pacing
    QK_PE = 1278   # 6 matmuls F=512
    VH_PE = 320    # 6 matmuls F=128
    PV_PE = 852    # 4 matmuls F=512
    OP_PE = 960    # 3x F=512 + 3x F=256

    # ---- emission helpers ----------------------------------------------
    qT, kT = [None] * 3, [None] * 3
    v_t = qkv.tile([128, NT, 384], BF16, tag="v")
    outnT = []
    for h in range(EH):
        outnT.append(qkv.tile([128, N], BF16, tag=f"outnT{h}", name=f"outnT{h}"))

    def proj_qk_chunk(m, name, cc, tag="pv", cols=None, ts_eng=None):
        bias = bq_t if name == "q" else bk_t
        dst = qT if name == "q" else kT
        out_t = dst[m]
        c0, c1 = cols if cols else (cc * 512, cc * 512 + 512)
        w = c1 - c0
        ps = pvp.tile([128, 512], F32, tag=tag, name="psq")
        for kblk in range(6):
            nc.tensor.matmul(
                ps[:, 0:w],
                w_t[name][kblk][:, ts(m, 128)],
                xt[:, kblk, c0:c1],
                start=(kblk == 0),
                stop=(kblk == 5),
            )
        (ts_eng or nc.vector).tensor_scalar(
            out=out_t[:, c0:c1],
            in0=ps[:, 0:w],
            scalar1=bias[:, m : m + 1],
            scalar2=None,
            op0=OP.add,
        )

    def proj_qk_start(m):
        # the first attn block runs k-half-major: it needs kT[0] cc0/cc1
        # and qT[0] cc0 first (x cols 0:1024 + wk + wq land ~6us); kT[0]
        # cc2/cc3 (x cols 1024:2048, ~8.5us) are emitted by the block0
        # mid-callback between its two exp pairs so the first two score
        # matmuls sit at the head of the PE queue
        for name in ("q", "k"):
            dst = qT if name == "q" else kT
            dst[m] = qkv.tile(
                [128, N], BF16, tag=f"{name}T{m}", name=f"{name}T{m}"
            )
        proj_qk_chunk(m, "k", 0, tag="pv")
        proj_qk_chunk(m, "k", 1, tag="pvacc")
        # block0 only needs q cols 0:128: a short chunk gets its bias-TS
        # (the last exp1 dependency) done ~1.3us sooner
        proj_qk_chunk(m, "q", 0, tag="pv", cols=(0, 128))
        proj_qk_chunk(m, "q", 0, tag="pvacc", cols=(128, 512))
        return [
            (QK_PE, 1, lambda m=m, cc=cc: proj_qk_chunk(m, "q", cc))
            for cc in (1, 2, 3)
        ]

    def proj_k_late(m):
        proj_qk_chunk(m, "k", 2, tag="pvacc")
        proj_qk_chunk(m, "k", 3, tag="pv")

    def proj_qk_chunks(m):
        # deadline: head m's first score matmuls are emitted at group 4*m
        for name in ("q", "k"):
            dst = qT if name == "q" else kT
            dst[m] = qkv.tile(
                [128, N], BF16, tag=f"{name}T{m}", name=f"{name}T{m}"
            )
        return [
            (QK_PE, 4 * m, lambda m=m, name=name, cc=cc: proj_qk_chunk(m, name, cc))
            for name in ("q", "k")
            for cc in range(4)
        ]

    def proj_v_chunks(hh):
        # deadline: pv(hh, 0) chunks pop during group (hh, 1)
        return [
            (VH_PE, 4 * hh + 1, lambda tb=tb, hh=hh: proj_v(tb, hh))
            for tb in range(NT)
        ]

    def proj_v(tb, hh):
        ps = pvp.tile([128, 512], F32, tag="pv", name="psv")
        for kblk in range(6):
            nc.tensor.matmul(
                ps[:, 0:128],
                xt[:, kblk, ts(tb, 128)],
                w_t["v"][kblk][:, ts(hh, 128)],
                start=(kblk == 0),
                stop=(kblk == 5),
            )
        nc.vector.tensor_tensor(
            out=v_t[:, tb, ts(hh, 128)],
            in0=ps[:, 0:128],
            in1=bv_t[:, ts(hh, 128)],
            op=OP.add,
        )

    def attn_block(h, g, b, dT, lamp_g, d1_g, chunked=False, mid=None):
        """One 128-q-row block: scores+exp for both sub-heads, then the
        differential combination and its transpose.  sub1 (E2) runs first
        so d2s/rec overlap sub0's exps and lamp is ready right after the
        last accum lands.  mid (first block only) orders the exps by
        k-half and emits the late kT chunks between the two exp pairs."""
        t1 = g * 4 + b
        dacc = small.tile([128, 4], F32, tag="dacc", name="dacc")
        e_t = {}
        for sub in (1, 0):
            e_t[sub] = epool.tile([128, N], BF16, tag="E", name="e")

        def exp_single(sub, half):
            ps = spool.tile([128, 1024], F32, tag="s", name="ps")
            for c2 in range(2):
                cc = half * 2 + c2
                nc.tensor.matmul(
                    ps[:, ts(c2, 512)],
                    qT[h][ts(sub, 64), ts(t1, 128)],
                    kT[h][ts(sub, 64), ts(cc, 512)],
                    start=True,
                    stop=True,
                )
            slot = (1 - sub) * 2 + half
            nc.scalar.activation(
                e_t[sub][:, ts(half, 1024)],
                ps[:],
                AF.Exp,
                accum_out=dacc[:, slot : slot + 1],
            )

        units = (
            [(0, 0), (1, 0), (0, 1), (1, 1)]
            if mid
            else [(1, 0), (1, 1), (0, 0), (0, 1)]
        )
        for ui, (sub, half) in enumerate(units):
            if mid and ui == 2:
                mid()
            exp_single(sub, half)
            if (sub, half) == (1, 1):
                # d2 and 1/d2 computed while sub0's exps stream
                d2s = small.tile([128, 1], F32, tag="d2s", name="d2s")
                nc.vector.tensor_tensor(
                    out=d2s[:], in0=dacc[:, 0:1], in1=dacc[:, 1:2], op=OP.add
                )
                rec = small.tile([128, 1], F32, tag="rec", name="rec")
                nc.vector.reciprocal(rec[:], d2s[:])
        d1 = d1_g[:, b : b + 1]
        nc.vector.tensor_tensor(
            out=d1, in0=dacc[:, 2:3], in1=dacc[:, 3:4], op=OP.add
        )
        nc.vector.tensor_scalar(
            out=lamp_g[:, b : b + 1],
            in0=rec[:],
            scalar1=d1,
            scalar2=lam_t[:],
            op0=OP.mult,
            op1=OP.mult,
        )
        # diff = lamp*E2 - E1 in ONE pass (the sign flip is folded into Wo
        # on the host); engine alternates DVE/Pool to split the load
        diff = dpool.tile([128, N], BF16, tag="diff", name="diff")
        if chunked:
            # final group: 512-col chunks, alternating engines, so the
            # per-b PV pipeline starts ~1.5us earlier on the endgame
            # critical path; for the very last block the XBARs split over
            # the sync + (now idle) ACT queues
            for qc in range(4):
                nc.vector.scalar_tensor_tensor(
                    out=diff[:, ts(qc, 512)],
                    in0=e_t[1][:, ts(qc, 512)],
                    scalar=lamp_g[:, b : b + 1],
                    in1=e_t[0][:, ts(qc, 512)],
                    op0=OP.mult,
                    op1=OP.subtract,
                )
                xq = nc.scalar if (b == 3 and qc % 2 == 1) else nc.sync
                xq.dma_start(
                    dT[:, qc * 4 : qc * 4 + 4, ts(b, 128)],
                    diff[:, ts(qc, 512)],
                    transpose=True,
                )
        elif (g * 4 + b) % 2 == 0:
            # one-pass STT on DVE (TensorScalarPtr is not a Pool opcode)
            nc.vector.scalar_tensor_tensor(
                out=diff[:],
                in0=e_t[1][:],
                scalar=lamp_g[:, b : b + 1],
                in1=e_t[0][:],
                op0=OP.mult,
                op1=OP.subtract,
            )
            nc.sync.dma_start(dT[:, :, ts(b, 128)], diff[:], transpose=True)
        else:
            # alternate blocks: TS on DVE, subtract on Pool (SBUF-only)
            nc.vector.tensor_scalar(
                out=diff[:],
                in0=e_t[1][:],
                scalar1=lamp_g[:, b : b + 1],
                scalar2=None,
                op0=OP.mult,
            )
            nc.gpsimd.tensor_tensor(
                out=diff[:], in0=diff[:], in1=e_t[0][:], op=OP.subtract
            )
            # One batched XBAR transpose: dT[k%128, k//128, b*128+q] = diff[q, k]
            nc.sync.dma_start(dT[:, :, ts(b, 128)], diff[:], transpose=True)

    def pv_chunk(h, dT, pv, j0):
        for j in range(j0, j0 + 4):
            nc.tensor.matmul(
                pv[:],
                v_t[:, j, ts(h, 128)],
                dT[:, j, :],
                start=(j == 0),
                stop=(j == NT - 1),
            )

    def tail_pre(h, g, dT, d1_g, pv, st):
        """Group tail, part 1 (no ACT work): evacuate PV, transpose to
        [q, dv] layout, and compute the rms Ln argument on DVE."""
        outT = onp.tile([128, 512], BF16, tag="outT", name="outT")
        nc.vector.tensor_copy(outT[:], pv[:])
        nat_sb = onp.tile([128, 4, 128], BF16, tag="natsb", name="natsb")
        nc.sync.dma_start(nat_sb[:], outT[:], transpose=True)
        sq = onp.tile([128, 512], F32, tag="sq", name="sq")
        nc.vector.tensor_tensor(
            out=sq[:],
            in0=nat_sb[:].rearrange("p a b -> p (a b)"),
            in1=nat_sb[:].rearrange("p a b -> p (a b)"),
            op=OP.mult,
        )
        zg = small.tile([128, 4], F32, tag="zg", name="zg")
        for bb in range(4):
            nc.vector.tensor_reduce(
                out=zg[:, bb : bb + 1],
                in_=sq[:, ts(bb, 128)],
                axis=mybir.AxisListType.X,
                op=OP.add,
            )
        # rms arg = zg/128 + eps*d1^2, computed as (zg + 128*eps*d1*d1)/128
        # so a single batched Ln (const bias) replaces 4 per-b Ln calls
        epsd = small.tile([128, 4], F32, tag="epsd", name="epsd")
        nc.vector.scalar_tensor_tensor(
            out=epsd[:],
            in0=d1_g[:],
            scalar=EPS * 128.0,
            in1=d1_g[:],
            op0=OP.mult,
            op1=OP.mult,
        )
        targ = small.tile([128, 4], F32, tag="targ", name="targ")
        nc.vector.tensor_tensor(out=targ[:], in0=zg[:], in1=epsd[:], op=OP.add)
        st["nat"] = nat_sb
        st["targ"] = targ

    def tail_act(h, g, st):
        """Group tail, part 2 (the only non-exp ACT work): batched Ln/Exp
        rsqrt, scale, and transpose back.  Emitted a block later than
        tail_pre so the Ln never queue-blocks the next group's exps."""
        nat_sb, targ = st["nat"], st["targ"]
        lz = small.tile([128, 4], F32, tag="lz", name="lz")
        nc.scalar.activation(lz[:], targ[:], AF.Ln, scale=1.0 / 128.0)
        rg = small.tile([128, 4], F32, tag="rg", name="rg")
        nc.scalar.activation(rg[:], lz[:], AF.Exp, scale=-0.5)
        onn = onp.tile([128, 4, 128], BF16, tag="onn", name="onn")
        for bb in range(4):
            nc.vector.tensor_scalar(
                out=onn[:, bb, :],
                in0=nat_sb[:, bb, :],
                scalar1=rg[:, bb : bb + 1],
                scalar2=None,
                op0=OP.mult,
            )
        nc.sync.dma_start(
            outnT[h][:, ts(g, 512)].rearrange("p (a b) -> p a b", a=4),
            onn[:].rearrange("p a b -> p (a b)"),
            transpose=True,
        )

    def per_b_tail(h, g, b, dT, d1_g, pv):
        tb = g * 4 + b
        for j in range(NT):
            nc.tensor.matmul(
                pv[:, ts(b, 128)],
                v_t[:, j, ts(h, 128)],
                dT[:, j, ts(b, 128)],
                start=(j == 0),
                stop=(j == NT - 1),
            )
        outTb = onp.tile([128, 128], BF16, tag="outTb", name="outTb")
        nc.vector.tensor_copy(outTb[:], pv[:, ts(b, 128)])
        natb = onp.tile([128, 128], BF16, tag="natb", name="natb")
        # tails for b2/b3 are emitted after the final exps, so their
        # transposes can ride the then-idle ACT queue instead of sync
        xq = nc.scalar if b >= 2 else nc.sync
        xq.dma_start(natb[:], outTb[:], transpose=True)
        scrb = onp.tile([128, 128], F32, tag="scrb", name="scrb")
        nc.vector.tensor_tensor(out=scrb[:], in0=natb[:], in1=natb[:], op=OP.mult)
        zgb = small.tile([128, 1], F32, tag="zgb", name="zgb")
        nc.vector.tensor_reduce(
            out=zgb[:], in_=scrb[:], axis=mybir.AxisListType.X, op=OP.add
        )
        d1b = d1_g[:, b : b + 1]
        epsb = small.tile([128, 1], F32, tag="epsb", name="epsb")
        nc.vector.scalar_tensor_tensor(
            out=epsb[:], in0=d1b, scalar=EPS * 128.0, in1=d1b, op0=OP.mult, op1=OP.mult
        )
        targb = small.tile([128, 1], F32, tag="targb", name="targb")
        nc.vector.tensor_tensor(out=targb[:], in0=zgb[:], in1=epsb[:], op=OP.add)
        lzb = small.tile([128, 1], F32, tag="lzb", name="lzb")
        nc.scalar.activation(lzb[:], targb[:], AF.Ln, scale=1.0 / 128.0)
        rgb = small.tile([128, 1], F32, tag="rgb", name="rgb")
        nc.scalar.activation(rgb[:], lzb[:], AF.Exp, scale=-0.5)
        onnb = onp.tile([128, 128], BF16, tag="onnb", name="onnb")
        nc.vector.tensor_scalar(
            out=onnb[:], in0=natb[:], scalar1=rgb[:], scalar2=None, op0=OP.mult
        )
        xq.dma_start(outnT[h][:, ts(tb, 128)], onnb[:], transpose=True)

    def outproj_tb(tb):
        ya = pvp.tile([128, 512], F32, tag="pv", name="ya")
        for hh in range(EH):
            nc.tensor.matmul(
                ya[:],
                outnT[hh][:, ts(tb, 128)],
                wo_t[hh][:, 0:512],
                start=(hh == 0),
                stop=(hh == EH - 1),
            )
        yb = pvp.tile([128, 512], F32, tag="pv", name="yb")
        for hh in range(EH):
            nc.tensor.matmul(
                yb[:, 0:256],
                outnT[hh][:, ts(tb, 128)],
                wo_t[hh][:, 512:768],
                start=(hh == 0),
                stop=(hh == EH - 1),
            )
        # y rides the Pool DMA queue: keeps its traffic off the
        # XBAR-carrying sync queue during the endgame (Pool itself cannot
        # read PSUM, so the evacuation copies stay on DVE)
        yt = ysb.tile([128, D], F32, tag="y", name="yt")
        nc.vector.tensor_copy(yt[:, 0:512], ya[:])
        nc.gpsimd.dma_start(y[ts(tb, 128), 0:512], yt[:, 0:512])
        nc.vector.tensor_copy(yt[:, 512:768], yb[:, 0:256])
        nc.gpsimd.dma_start(y[ts(tb, 128), 512:768], yt[:, 512:768])

    def outproj_fast(tb):
        """Endgame out-projection: after the last exp the score psum ring
        is free, so ya/yb go to one 2-bank spool tile (no slot chain)."""
        t = spool.tile([128, 1024], F32, tag="s", name="opf")
        for hh in range(EH):
            nc.tensor.matmul(
                t[:, 0:512],
                outnT[hh][:, ts(tb, 128)],
                wo_t[hh][:, 0:512],
                start=(hh == 0),
                stop=(hh == EH - 1),
            )
        for hh in range(EH):
            nc.tensor.matmul(
                t[:, 512:768],
                outnT[hh][:, ts(tb, 128)],
                wo_t[hh][:, 512:768],
                start=(hh == 0),
                stop=(hh == EH - 1),
            )
        yt = ysb.tile([128, D], F32, tag="y", name="ytf")
        nc.vector.tensor_copy(yt[:], t[:, 0:768])
        nc.gpsimd.dma_start(y[ts(tb, 128), :], yt[:])

    # ---- emission schedule ---------------------------------------------
    # Work deques of (pe_cost_ns, thunk):
    #   bg      — projection chunks (q rest, per-head v, later heads' q/k)
    #   pending — prior groups' PV chunks, tail_pre/tail_act, out-proj tiles
    # tail_act directly follows tail_pre: by the time ACT drains the 4.9us
    # of exps queued ahead of its Ln, the DVE rms-arg chain (~1.5us) has
    # long finished, so the Ln never head-of-line-blocks the exp stream.
    # Pops happen AFTER a block's score matmuls are enqueued, within a PE
    # budget, so ACT (exp, the critical engine) never waits on queued PE
    # work.  At b==3 only PE-free units pop: the next group's first score
    # matmuls must reach the PE queue with nothing in front of them.
    def pop_budget(q, budget):
        spent = 0
        while q:
            pe = q[0][0]
            if pe and spent + pe > budget:
                break
            spent += pe
            q.pop(0)[-1]()

    def pop_free(q):
        while q and q[0][0] == 0:
            q.pop(0)[-1]()

    def pop_overdue(q, gi):
        # correctness net: emission order defines the dependency graph, so
        # every projection chunk MUST be emitted before the first score
        # matmul that reads it; budgets are tuned so this never fires
        while q and q[0][1] <= gi:
            q.pop(0)[-1]()

    bg = (
        proj_qk_start(0)
        + proj_v_chunks(0)
        + proj_qk_chunks(1)
        + proj_v_chunks(1)
        + proj_qk_chunks(2)
        + proj_v_chunks(2)
    )
    late_w_dmas()
    # per-head (bg, bg@b2, pending, pending@b2) budgets: h0/h1 front-load
    # the next head's qk chunks (hard deadline: that head's first scores),
    # h2 drains pv+tails+outproj (batch ~10.6us/group); b2 tapers so
    # popped work never delays the next group's first score matmuls (b2's
    # taper still fits one 1278ns qk chunk so the deque keeps moving)
    BUDGETS = {
        0: (2200, 1400, 1400, 700),
        1: (2000, 1400, 2000, 1000),
        2: (1200, 600, 3800, 2600),
    }
    pending = []
    for h in range(EH):
        for g in range(4):
            first = h == 0 and g == 0
            last = h == EH - 1 and g == 3
            pop_overdue(bg, h * 4 + g)
            dT = dtp.tile([128, NT, 512], BF16, tag="dT", name="dT")
            lamp_g = small.tile([128, 4], F32, tag="lamp", name="lamp")
            d1_g = small.tile([128, 4], F32, tag="d1g", name="d1g")
            pv = pvp.tile([128, 512], F32, tag="pvacc", name="pv")
            for b in range(4):
                attn_block(
                    h, g, b, dT, lamp_g, d1_g,
                    chunked=last,
                    mid=(lambda: proj_k_late(0)) if first and b == 0 else None,
                )
                if last:
                    # final group: per-b pipeline, emitted 1 block late so
                    # the XBAR'd dT is ready when its PV matmuls dequeue
                    pop_budget(pending, 2000 if b == 2 else 2400)
                    if b >= 1:
                        per_b_tail(h, g, b - 1, dT, d1_g, pv)
                    continue
                bgb, bgb2, pb, pb2 = BUDGETS[h]
                if b == 3:
                    pop_free(pending)
                    if h == EH - 1:
                        pop_budget(pending, 1800)
                    continue
                if b == 2:
                    pop_budget(bg, bgb2)
                    pop_budget(pending, pb2)
                else:
                    pop_budget(bg, 4600 if first else bgb)
                    pop_budget(pending, pb)

            if last:
                # ops for tb12/13 are dep-free now and fill the dT(b3)
                # XBAR wait; op14 runs under b3's tail chain; leftover
                # pending ops (psum-slot chained) drain dead last
                outproj_fast(g * 4)
                outproj_fast(g * 4 + 1)
                per_b_tail(h, g, 3, dT, d1_g, pv)
                outproj_fast(g * 4 + 2)
                outproj_fast(g * 4 + 3)
                while pending:
                    pending.pop(0)[-1]()
                continue

            st = {}
            pending += [
                (PV_PE, lambda h=h, dT=dT, pv=pv, j0=j0: pv_chunk(h, dT, pv, j0))
                for j0 in (0, 4, 8, 12)
            ] + [
                (0, lambda h=h, g=g, dT=dT, d1_g=d1_g, pv=pv, st=st: tail_pre(
                    h, g, dT, d1_g, pv, st)),
                (0, lambda h=h, g=g, st=st: tail_act(h, g, st)),
            ]
            if h == EH - 1:
                # pseudo-cost 1800 (> real 960): the ya/yb psum slot is
                # held ~2.5us per op, so at most one op pops per block —
                # a second would head-of-line-block PE on the slot wait
                pending += [
                    (1800, lambda tb=tb: outproj_tb(tb))
                    for tb in range(g * 4, g * 4 + 4)
                ]
    for item in pending + bg:
        item[-1]()


def build_kernel():
    nc = bacc.Bacc("TRN2", target_bir_lowering=False, debug=False, num_devices=1)
    aps = [
        nc.dram_tensor("xT", [D, N], BF16, kind="ExternalInput").ap(),
        nc.dram_tensor("wq", [D, 384], BF16, kind="ExternalInput").ap(),
        nc.dram_tensor("wk", [D, 384], BF16, kind="ExternalInput").ap(),
        nc.dram_tensor("wv", [D, 384], BF16, kind="ExternalInput").ap(),
        nc.dram_tensor("wo", [384, D], BF16, kind="ExternalInput").ap(),
        nc.dram_tensor("bq", [128, 3], F32, kind="ExternalInput").ap(),
        nc.dram_tensor("bk", [128, 3], F32, kind="ExternalInput").ap(),
        nc.dram_tensor("bv128", [128, 384], F32, kind="ExternalInput").ap(),
        nc.dram_tensor("lam128", [128, 1], F32, kind="ExternalInput").ap(),
        nc.dram_tensor("y", [N, D], F32, kind="ExternalOutput").ap(),
    ]
    with tile.TileContext(nc, trace_sim=False) as tc:
        with ExitStack() as ctx:
            _body(ctx, tc, *aps)
    nc.compile()
    return nc


def make_in_maps(x, Wq, bqv, Wk, bkv, Wv, bvv, lambda_q1, lambda_k1,
                 lambda_q2, lambda_k2, norm_weight):
    scaling = HD ** -0.5
    lam1 = np.exp(np.sum(lambda_q1.astype(np.float64) * lambda_k1.astype(np.float64)))
    lam2 = np.exp(np.sum(lambda_q2.astype(np.float64) * lambda_k2.astype(np.float64)))
    lam = np.float32(lam1 - lam2 + LAMBDA_INIT)

    in_maps = []
    for c in range(NCORES):
        b = c // 2
        hs = 3 * (c % 2)
        cols = slice(128 * hs, 128 * (hs + 3))
        in_maps.append(
            {
                "xT": np.ascontiguousarray(x[b].T).astype(BF),
                "wq": np.ascontiguousarray(Wq[:, cols] * scaling).astype(BF),
                "wk": np.ascontiguousarray(Wk[:, cols]).astype(BF),
                "wv": np.ascontiguousarray(Wv[:, cols]).astype(BF),
                "wo": None,  # filled below (0.2*norm_weight folded in)
                "bq": np.ascontiguousarray(
                    (bqv[cols] * scaling).reshape(3, 128).T
                ).astype(np.float32),
                "bk": np.ascontiguousarray(bkv[cols].reshape(3, 128).T).astype(
                    np.float32
                ),
                "bv128": np.ascontiguousarray(
                    np.broadcast_to(bvv[cols], (128, 384))
                ).astype(np.float32),
                "lam128": np.full((128, 1), lam, np.float32),
            }
        )
    return in_maps, lam


def fold_wo(Wo, norm_weight):
    # norm_weight is a per-(2*HD)-lane diagonal scale right before Wo:
    # out_n @ (diag(0.2*wn) @ Wo) == (out_n*0.2*wn) @ Wo.  Folded on the
    # host, NEGATED: the kernel computes diff = lamp*E2 - E1 (one STT op)
    # which is -1x the reference's differential.
    return np.tile(-(1.0 - LAMBDA_INIT) * norm_weight, 6).reshape(768, 1) * Wo


_NC_CACHE = {}


def kernel(**inputs):
    x = np.asarray(inputs["x"], np.float32)
    Wq = np.asarray(inputs["Wq"], np.float32)
    bq = np.asarray(inputs["bq"], np.float32)
    Wk = np.asarray(inputs["Wk"], np.float32)
    bk = np.asarray(inputs["bk"], np.float32)
    Wv = np.asarray(inputs["Wv"], np.float32)
    bv = np.asarray(inputs["bv"], np.float32)
    Wo = np.asarray(inputs["Wo"], np.float32)
    bo = np.asarray(inputs["bo"], np.float32)
    norm_weight = np.asarray(inputs["norm_weight"], np.float32)

    in_maps, _lam = make_in_maps(
        x, Wq, bq, Wk, bk, Wv, bv,
        np.asarray(inputs["lambda_q1"], np.float32),
        np.asarray(inputs["lambda_k1"], np.float32),
        np.asarray(inputs["lambda_q2"], np.float32),
        np.asarray(inputs["lambda_k2"], np.float32),
        norm_weight,
    )
    wos = fold_wo(Wo, norm_weight)
    for c in range(NCORES):
        hs = 3 * (c % 2)
        cols = slice(128 * hs, 128 * (hs + 3))
        in_maps[c]["wo"] = np.ascontiguousarray(wos[cols, :]).astype(BF)

    if "nc" not in _NC_CACHE:
        _NC_CACHE["nc"] = build_kernel()
    nc = _NC_CACHE["nc"]

    res = bass_utils.run_bass_kernel_spmd(nc, in_maps, core_ids=list(range(NCORES)))

    out = np.empty((B, N, D), np.float32)
    for b in range(B):
        out[b] = res.results[2 * b]["y"] + res.results[2 * b + 1]["y"] + bo
    return out
